# revision 1
# baseline (speedup 1.0000x reference)
"""GQA kernel for Trainium2, 8-way tensor-parallel over kv heads.

Problem (hardcoded): B=1, S=2048, D_MODEL=2048, HQ=32, HKV=8, DK=64, causal.
Sharding: core h owns kv head h and its 4 query heads. Weights are sliced on
host; x is replicated. Each core computes a partial y (its heads projected
through its slice of wo); host sums the 8 partials and adds bo.

On-chip dataflow per core (all matmuls bf16 with fp32 PSUM accumulation):
  x --DMA--> SBUF --PE transpose--> XT[d, s] (bf16)
  QKVT[384, S] projection, evacuated via ACT with fused bias add
  scoresT[j, i] = K_j.T-block @ Q_i (per head, causal strips only)
  expT = exp(0.125 * scoresT)  (no max subtraction: |scaled scores| < ~6)
  out_aug[65, i] = V_aug.T @ expT, V_aug has a ones column -> row 64 = sums
  normalize via reciprocal + K=1 broadcast matmul, then y = outT.T @ woT

Emission is software-pipelined: engines execute their streams in order, so
x-loads/transposes for i-tile ti+1 are emitted between attention head-groups
of ti, and QKV(ti+1) before the output projection F(ti). This keeps PE/ACT/
DVE/DMA all fed; per the cost model the kernel is PE-bound.
"""

import numpy as np

S = 2048
D = 2048
HQ, HKV, GRP, DK = 32, 8, 4, 64
QF = GRP * DK            # 256 query features per core
KVF = 2 * DK             # 128 (k then v)
NF = QF + KVF            # 384 projected features per core
N_CORES = 8
NT = S // 128            # 16 s-blocks / d-tiles
NI = S // 512            # 4 i-tiles

_cache = {}


def _build(debug_dumps=False):
    import concourse.bass as bass
    import concourse.mybir as mybir
    from concourse import bacc, tile
    from concourse.masks import make_identity
    from contextlib import ExitStack

    f32 = mybir.dt.float32
    bf16 = mybir.dt.bfloat16
    Exp = mybir.ActivationFunctionType.Exp
    Ident = mybir.ActivationFunctionType.Identity

    nc = bacc.Bacc(
        "TRN2",
        target_bir_lowering=False,
        debug=False,
        enable_asserts=False,
        num_devices=N_CORES,
    )
    x_d = nc.dram_tensor("x", [S, D], f32, kind="ExternalInput").ap()
    wqkv_d = nc.dram_tensor("wqkv", [NF, D], f32, kind="ExternalInput").ap()
    bqkv_d = nc.dram_tensor("bqkv", [NF, 1], f32, kind="ExternalInput").ap()
    wo_d = nc.dram_tensor("wo", [D, QF], f32, kind="ExternalInput").ap()
    y_d = nc.dram_tensor("y", [S, D], f32, kind="ExternalOutput").ap()
    dbg = {}
    if debug_dumps:
        for nm, shp in [("d_XT", [128, NT * S]), ("d_qkvT0", [128, S]),
                        ("d_qkvT1", [128, S]), ("d_qkvT2", [128, S]),
                        ("d_kdup", [128, S]), ("d_vaug", [128, NT * (DK + 1)]),
                        ("d_outT0", [128, S]), ("d_outT1", [128, S])]:
            dbg[nm] = nc.dram_tensor(nm, shp, bf16, kind="ExternalOutput").ap()

    with tile.TileContext(nc) as tc, ExitStack() as ctx:
        const = ctx.enter_context(tc.tile_pool(name="const", bufs=1))
        stg = ctx.enter_context(tc.tile_pool(name="stg", bufs=3))
        expp = ctx.enter_context(tc.tile_pool(name="expp", bufs=8))
        ysbp = ctx.enter_context(tc.tile_pool(name="ysbp", bufs=4))
        rcp = ctx.enter_context(tc.tile_pool(name="rcp", bufs=2))
        tmpp = ctx.enter_context(tc.tile_pool(name="tmpp", bufs=2))
        # PSUM budget (8 banks): ps_mm 2 (transposes + QKV + F, [128,512]f32)
        # + ps_sc 2x [128,1024]f32 = 4 (paired score strips) + ps_aug 2
        # (AV accumulator + broadcast tile)
        ps_mm = ctx.enter_context(tc.tile_pool(name="ps_mm", bufs=2, space="PSUM"))
        ps_sc = ctx.enter_context(tc.tile_pool(name="ps_sc", bufs=2, space="PSUM"))
        ps_aug = ctx.enter_context(tc.tile_pool(name="ps_aug", bufs=2, space="PSUM"))

        # ---- constants ----
        ident32 = const.tile([128, 128], f32)
        make_identity(nc, ident32)
        ident_bf = const.tile([128, 128], bf16)
        make_identity(nc, ident_bf)
        # ones row at partition 64 to match aug's sums row (engines cannot
        # shift partitions, so the whole recip chain stays on row 64)
        ones_bf = const.tile([65, 128], bf16)
        nc.gpsimd.memset(ones_bf, 1.0)

        # ---- persistent SBUF ----
        XT = const.tile([128, NT * S], bf16)       # [d%128, dtile, s]
        XTv = XT.rearrange("p (t s) -> p t s", t=NT)
        wT = const.tile([128, NT * NF], bf16)      # [d%128, dtile, feat]
        wTv = wT.rearrange("p (t f) -> p t f", t=NT)
        woT = const.tile([128, 2 * D], bf16)       # [f%128, ftile, m]
        woTv = woT.rearrange("p (t m) -> p t m", t=2)
        qkvT = [const.tile([128, S], bf16, name=f"qkvT{i}") for i in range(3)]
        # K duplicated at partition base 64 (odd heads read Q at base 64, and
        # matmul requires lhsT/rhs on the same base partition)
        kdup = const.tile([128, S], bf16)
        outT = [const.tile([128, S], bf16, name=f"outT{i}") for i in range(2)]
        v_aug = const.tile([128, NT * (DK + 1)], bf16)
        nc.gpsimd.memset(v_aug, 1.0)               # ones col survives at 64::65
        btiles = [const.tile([128, 1], f32, name=f"btile{i}") for i in range(3)]

        # ---- stage emitters ----
        def emit_wqkv_stage():
            for mi in range(3):
                wstg = stg.tile([128, D], f32, tag="stg", name="wstg")
                nc.sync.dma_start(out=wstg, in_=wqkv_d[mi * 128:(mi + 1) * 128, :])
                for dq in range(4):
                    psw = ps_mm.tile([128, 512], f32, tag="ps_mm", name="psw")
                    for q in range(4):
                        dt = 4 * dq + q
                        nc.tensor.transpose(
                            psw[:, q * 128:(q + 1) * 128],
                            wstg[:, dt * 128:(dt + 1) * 128],
                            ident32,
                        )
                    nc.scalar.copy(
                        wTv[:, 4 * dq:4 * dq + 4, mi * 128:(mi + 1) * 128],
                        psw.rearrange("p (a b) -> p a b", a=4),
                    )
                nc.sync.dma_start(
                    out=btiles[mi], in_=bqkv_d[mi * 128:(mi + 1) * 128, :])

        def emit_wo_block(mb):
            wostg = stg.tile([128, QF], f32, tag="stg", name="wostg")
            nc.sync.dma_start(out=wostg, in_=wo_d[mb * 128:(mb + 1) * 128, :])
            psw = ps_mm.tile([128, 512], f32, tag="ps_mm", name="psw")
            for ft in range(2):
                nc.tensor.transpose(
                    psw[:, ft * 128:(ft + 1) * 128],
                    wostg[:, ft * 128:(ft + 1) * 128],
                    ident32,
                )
            nc.scalar.copy(
                woTv[:, 0:2, mb * 128:(mb + 1) * 128],
                psw.rearrange("p (a b) -> p a b", a=4)[:, 0:2, :],
            )

        def emit_B_block(sb):
            # load 128 rows of x, PE-transpose into XT columns
            xstg = stg.tile([128, D], f32, tag="stg", name="xstg")
            nc.sync.dma_start(out=xstg, in_=x_d[sb * 128:(sb + 1) * 128, :])
            for dq in range(4):
                psx = ps_mm.tile([128, 512], f32, tag="ps_mm", name="psx")
                for q in range(4):
                    dt = 4 * dq + q
                    nc.tensor.transpose(
                        psx[:, q * 128:(q + 1) * 128],
                        xstg[:, dt * 128:(dt + 1) * 128],
                        ident32,
                    )
                eng = nc.vector.tensor_copy if dq % 2 == 0 else nc.scalar.copy
                eng(
                    XTv[:, 4 * dq:4 * dq + 4, sb * 128:(sb + 1) * 128],
                    psx.rearrange("p (a b) -> p a b", a=4),
                )

        def emit_C(ni):
            # QKV projection for s-columns of i-tile ni
            for mi in range(3):
                psq = ps_mm.tile([128, 512], f32, tag="ps_mm", name="psq")
                for dt in range(NT):
                    nc.tensor.matmul(
                        psq,
                        lhsT=wTv[:, dt, mi * 128:(mi + 1) * 128],
                        rhs=XTv[:, dt, ni * 512:(ni + 1) * 512],
                        start=(dt == 0),
                        stop=(dt == NT - 1),
                    )
                nc.scalar.activation(
                    qkvT[mi][:, ni * 512:(ni + 1) * 512],
                    psq, Ident, bias=btiles[mi], scale=1.0,
                )
            nc.sync.dma_start(
                out=kdup[64:128, ni * 512:(ni + 1) * 512],
                in_=qkvT[2][0:DK, ni * 512:(ni + 1) * 512],
            )

        def emit_D(ni):
            # V blocks transposed into V_aug
            for bj in range(4 * ni, 4 * ni + 4):
                psv = ps_mm.tile([128, 512], bf16, tag="ps_mm", name="psv")
                nc.tensor.transpose(
                    psv[:, 0:DK],
                    qkvT[2][64:128, bj * 128:(bj + 1) * 128],
                    ident_bf[64:128, 64:128],
                )
                nc.vector.tensor_copy(
                    v_aug[:, bj * (DK + 1): bj * (DK + 1) + DK], psv[:, 0:DK])

        def emit_E_scores_av(ti, g):
            # strips are processed in pairs sharing one [128, 1024] PSUM tile
            # (2 banks, one matmul per bank) so exp runs as a single ACT op
            # per pair -- halves ACT fixed overhead
            qrow = (g % 2) * 64
            aug = ps_aug.tile([DK + 1, 512], f32, tag="ps_aug", name="aug")
            ksrc = qkvT[2] if qrow == 0 else kdup
            nstrip = 4 * ti + 4
            for p in range(nstrip // 2):
                sc = ps_sc.tile([128, 1024], f32, tag="ps_sc", name="sc")
                halves = []
                lo = 0
                for half in range(2):
                    bj = 2 * p + half
                    # causal: strip bj needs only columns i >= bj*128, i.e.
                    # free offset >= 128k within this i-tile (k = bj - 4*ti)
                    k = bj - 4 * ti
                    off = 128 * k if k > 0 else 0
                    width = 512 - off
                    # packed back-to-back: [lo, lo+width) never crosses a
                    # PSUM bank boundary for the widths that occur here
                    halves.append((bj, k, off, width, lo))
                    nc.tensor.matmul(
                        sc[:, lo:lo + width],
                        lhsT=ksrc[qrow:qrow + DK, bj * 128:(bj + 1) * 128],
                        rhs=qkvT[g // 2][qrow:qrow + 64,
                                         ti * 512 + off:(ti + 1) * 512],
                        start=True, stop=True,
                    )
                    lo += width
                expT = expp.tile([128, 1024], bf16, tag="expp", name="expT")
                nc.scalar.activation(expT[:, 0:lo], sc[:, 0:lo],
                                     Exp, scale=0.125)
                for (bj, k, off, width, lo) in halves:
                    if k >= 0:
                        # zero the j > i triangle (keep where col >= row)
                        nc.gpsimd.affine_select(
                            out=expT[:, lo:lo + width],
                            in_=expT[:, lo:lo + width],
                            compare_op=mybir.AluOpType.is_ge,
                            fill=0.0, base=0,
                            pattern=[[1, width]], channel_multiplier=-1,
                        )
                    nc.tensor.matmul(
                        aug[:, off:512],
                        lhsT=v_aug[:, bj * (DK + 1): (bj + 1) * (DK + 1)],
                        rhs=expT[:, lo:lo + width],
                        start=(bj == 0),
                        stop=(bj == nstrip - 1),
                    )
            return aug

        def emit_E_norm(ti, g, aug):
            qrow = (g % 2) * 64
            recip32 = rcp.tile([65, 512], f32, tag="recip32", name="recip32")
            nc.vector.reciprocal(recip32[DK:DK + 1, :], aug[DK:DK + 1, :])
            recipbf = rcp.tile([65, 512], bf16, tag="recipbf", name="recipbf")
            nc.vector.tensor_copy(recipbf[DK:DK + 1, :], recip32[DK:DK + 1, :])
            bc = ps_aug.tile([128, 512], f32, tag="ps_aug", name="bc")
            nc.tensor.matmul(bc, lhsT=ones_bf[DK:DK + 1, :],
                             rhs=recipbf[DK:DK + 1, :], start=True, stop=True)
            tmp = tmpp.tile([64, 512], f32, tag="tmpp", name="tmp")
            nc.vector.tensor_copy(tmp, aug[0:DK, :])
            if qrow == 0:
                nc.vector.tensor_mul(
                    outT[g // 2][0:64, ti * 512:(ti + 1) * 512],
                    tmp, bc[0:DK, :])
            else:
                # engines cannot shift partitions: stage at base 0, DMA up
                nstage = tmpp.tile([64, 512], bf16, tag="nstage", name="nstage")
                nc.vector.tensor_mul(nstage, tmp, bc[0:DK, :])
                nc.sync.dma_start(
                    out=outT[g // 2][64:128, ti * 512:(ti + 1) * 512],
                    in_=nstage,
                )

        def emit_F_block(sb):
            for mi2 in range(4):
                psy = ps_mm.tile([128, 512], f32, tag="ps_mm", name="psy")
                for ft in range(2):
                    nc.tensor.matmul(
                        psy,
                        lhsT=outT[ft][:, sb * 128:(sb + 1) * 128],
                        rhs=woTv[:, ft, mi2 * 512:(mi2 + 1) * 512],
                        start=(ft == 0),
                        stop=(ft == 1),
                    )
                ysb = ysbp.tile([128, 512], f32, tag="ysbp", name="ysb")
                if (sb + mi2) % 2 == 0:
                    nc.vector.tensor_copy(ysb, psy)
                else:
                    nc.scalar.copy(ysb, psy)
                nc.sync.dma_start(
                    out=y_d[sb * 128:(sb + 1) * 128,
                            mi2 * 512:(mi2 + 1) * 512],
                    in_=ysb,
                )

        # ---- pipelined schedule ----
        emit_B_block(0)
        emit_B_block(1)
        emit_wqkv_stage()
        emit_B_block(2)
        emit_B_block(3)
        emit_C(0)
        emit_D(0)
        for ti in range(NI):
            for g in range(GRP):
                aug = emit_E_scores_av(ti, g)
                if ti < NI - 1:
                    emit_B_block(4 * (ti + 1) + g)
                if ti == 0:
                    for mb in range(4 * g, 4 * g + 4):
                        emit_wo_block(mb)
                else:
                    # output projection of the previous i-tile, interleaved
                    # here to fill the attention phase's PE/DVE idle time
                    emit_F_block(4 * (ti - 1) + g)
                emit_E_norm(ti, g, aug)
            if ti < NI - 1:
                emit_C(ti + 1)
                emit_D(ti + 1)
        for sb in range(4 * (NI - 1), 4 * NI):
            emit_F_block(sb)

        if debug_dumps:
            for nm, src in [("d_XT", XT), ("d_qkvT0", qkvT[0]),
                            ("d_qkvT1", qkvT[1]), ("d_qkvT2", qkvT[2]),
                            ("d_kdup", kdup), ("d_vaug", v_aug),
                            ("d_outT0", outT[0]), ("d_outT1", outT[1])]:
                nc.sync.dma_start(out=dbg[nm], in_=src)

    nc.compile()
    return nc


def _get_nc():
    if "nc" not in _cache:
        _cache["nc"] = _build()
    return _cache["nc"]


def _in_maps(x, wq, bq, wk, bk, wv, bv, wo):
    maps = []
    for h in range(N_CORES):
        qs = slice(h * QF, (h + 1) * QF)
        ks = slice(h * DK, (h + 1) * DK)
        wqkv = np.concatenate([wq[qs], wk[ks], wv[ks]], axis=0).astype(np.float32)
        bqkv = np.concatenate([bq[qs], bk[ks], bv[ks]], axis=0).astype(np.float32)
        maps.append({
            "x": np.ascontiguousarray(x, dtype=np.float32),
            "wqkv": np.ascontiguousarray(wqkv),
            "bqkv": np.ascontiguousarray(bqkv[:, None]),
            "wo": np.ascontiguousarray(wo[:, qs], dtype=np.float32),
        })
    return maps


def _run(inputs, trace=False, tmpdir=None):
    from concourse.bass_utils import run_bass_kernel_spmd

    nc = _get_nc()
    x = np.asarray(inputs["x"])[0]
    maps = _in_maps(
        x,
        np.asarray(inputs["wq"]), np.asarray(inputs["bq"]),
        np.asarray(inputs["wk"]), np.asarray(inputs["bk"]),
        np.asarray(inputs["wv"]), np.asarray(inputs["bv"]),
        np.asarray(inputs["wo"]),
    )
    res = run_bass_kernel_spmd(
        nc, maps, list(range(N_CORES)), trace=trace, tmpdir=tmpdir
    )
    y = np.zeros((S, D), dtype=np.float32)
    for i in range(N_CORES):
        y += res.results[i]["y"]
    y += np.asarray(inputs["bo"])[None, :]
    return y[None], res


def kernel(**inputs):
    y, _ = _run(inputs, trace=False)
    return y.astype(np.float32)



# revision 11
# speedup vs baseline: 1.3815x; 1.3815x over previous
"""GQA kernel for Trainium2, 8-way tensor-parallel over kv heads.

Problem (hardcoded): B=1, S=2048, D_MODEL=2048, HQ=32, HKV=8, DK=64, causal.
Sharding: core h owns kv head h and its 4 query heads. Weights are sliced,
transposed and cast to bf16 on host; x is replicated (transposed bf16). Each
core computes a partial y (its heads projected through its slice of wo); host
sums the 8 partials and adds bo.

On-chip dataflow per core (all matmuls bf16 with fp32 PSUM accumulation):
  xT, wqkvT, woT arrive pre-transposed -> no PE transposes for x/weights
  qkvT[f, s] projection, evacuated via DVE with fused per-partition bias
  scores: heads processed in pairs (g_even at array rows 0-63, g_odd at rows
    64-127 via duplicated K) -> the two 64-contraction matmuls run
    CONCURRENTLY in separate PE row groups (tile_position auto-derived)
  expT = exp(0.125 * scoresT) on ACT, causal strips only; diagonal blocks
    masked by gpsimd affine_select on the leading 128 columns only
  AV: col-tiled pair -- head g_even -> PSUM partitions 0-63, g_odd -> 64-127
    of one bank (concurrent), V is 64-wide; denominators via ones-column
    matmuls into partitions 0 / 32 of a second bank (concurrent pair)
  normalize: reciprocal_approx_fast + tiny broadcast matmuls + one DVE mul
  y = outT.T @ woT per 128-row block, interleaved into attention slack

Emission is software-pipelined with a filler queue: F(ti-1) and QKV(ti+1)
matmul chunks are popped between attention strips so PE stays busy while ACT
computes exp (ACT paces the attention phase).
"""

from collections import deque

import numpy as np

S = 2048
D = 2048
HQ, HKV, GRP, DK = 32, 8, 4, 64
QF = GRP * DK            # 256 query features per core
NF = QF + 2 * DK         # 384 projected features per core
N_CORES = 8
NT = S // 128            # 16 d-tiles
NI = S // 512            # 4 i-tiles

_cache = {}


def _build(debug_dumps=False):
    import concourse.bass as bass
    import concourse.mybir as mybir
    from concourse import bacc, tile
    from concourse.masks import make_identity
    from contextlib import ExitStack

    f32 = mybir.dt.float32
    bf16 = mybir.dt.bfloat16
    Exp = mybir.ActivationFunctionType.Exp

    nc = bacc.Bacc(
        "TRN2",
        target_bir_lowering=False,
        debug=False,
        enable_asserts=False,
        num_devices=N_CORES,
    )
    # host layouts (see _in_maps): xt[ni*128+p, dt*512+s'] = x[ni*512+s', dt*128+p]
    xt_d = nc.dram_tensor("xt", [NI * 128, NT * 512], bf16, kind="ExternalInput").ap()
    wt_d = nc.dram_tensor("wt", [128, NT * NF], bf16, kind="ExternalInput").ap()
    wo_d = nc.dram_tensor("wob", [128, 2 * D], bf16, kind="ExternalInput").ap()
    bq_d = nc.dram_tensor("bq", [NF, 1], f32, kind="ExternalInput").ap()
    y_d = nc.dram_tensor("y", [S, D], f32, kind="ExternalOutput").ap()
    dbg = {}
    if debug_dumps:
        for nm, shp in [("d_qkvT0", [128, S]), ("d_qkvT1", [128, S]),
                        ("d_qkvT2", [128, S]), ("d_kdup", [128, S]),
                        ("d_vS", [128, NT * DK]), ("d_outT0", [128, S]),
                        ("d_outT1", [128, S])]:
            dbg[nm] = nc.dram_tensor(nm, shp, bf16, kind="ExternalOutput").ap()
        for nm, shp in [("d_aug", [128, 512]), ("d_dn", [33, 512]),
                        ("d_bcS", [128, 512])]:
            dbg[nm] = nc.dram_tensor(nm, shp, f32, kind="ExternalOutput").ap()

    with tile.TileContext(nc) as tc, ExitStack() as ctx:
        const = ctx.enter_context(tc.tile_pool(name="const", bufs=1))
        expp = ctx.enter_context(tc.tile_pool(name="expp", bufs=4))
        ysbp = ctx.enter_context(tc.tile_pool(name="ysbp", bufs=4))
        rcpp = ctx.enter_context(tc.tile_pool(name="rcpp", bufs=2))
        # PSUM (8 banks): sc 2x[128,1024]=4, av 1, dn/bc 1, qkv 1, f/vt 1
        ps_sc = ctx.enter_context(tc.tile_pool(name="ps_sc", bufs=2, space="PSUM"))
        ps_av = ctx.enter_context(tc.tile_pool(name="ps_av", bufs=1, space="PSUM"))
        ps_dn = ctx.enter_context(tc.tile_pool(name="ps_dn", bufs=1, space="PSUM"))
        ps_qkv = ctx.enter_context(tc.tile_pool(name="ps_qkv", bufs=1, space="PSUM"))
        ps_f = ctx.enter_context(tc.tile_pool(name="ps_f", bufs=1, space="PSUM"))

        # ---- constants ----
        ident_bf = const.tile([128, 128], bf16)
        make_identity(nc, ident_bf)
        onescol = const.tile([128, 1], bf16)
        nc.gpsimd.memset(onescol, 1.0)
        onesB = const.tile([33, 64], bf16)
        nc.gpsimd.memset(onesB, 1.0)

        # ---- persistent SBUF ----
        XT = const.tile([128, NI * NT * 512], bf16)
        XTv = XT.rearrange("p (n t s) -> p n t s", n=NI, t=NT)
        wT = const.tile([128, NT * NF], bf16)
        wTv = wT.rearrange("p (t f) -> p t f", t=NT)
        woT = const.tile([128, 2 * D], bf16)
        woTv = woT.rearrange("p (t m) -> p t m", t=2)
        # qkvT[0] = Q heads g0|g1 (rows 0-63|64-127), [1] = g2|g3, [2] = K|V
        qkvT = [const.tile([128, S], bf16, name=f"qkvT{i}") for i in range(3)]
        kdup = const.tile([128, S], bf16)           # K duplicated at rows 64-127
        vS = const.tile([128, NT * DK], bf16)       # V as [s%128, strip, f]
        vSv = vS.rearrange("p (j f) -> p j f", j=NT)
        outT = [const.tile([128, S], bf16, name=f"outT{i}") for i in range(2)]
        btl = [const.tile([128, 1], f32, name=f"btl{i}") for i in range(3)]

        # ---- upfront DMAs (x slabs ordered by first use) ----
        nc.sync.dma_start(out=XTv[:, 0], in_=xt_d[0:128, :])
        nc.sync.dma_start(out=wT, in_=wt_d)
        for mi in range(3):
            nc.sync.dma_start(out=btl[mi], in_=bq_d[mi * 128:(mi + 1) * 128, :])
        nc.sync.dma_start(out=XTv[:, 1], in_=xt_d[128:256, :])
        nc.sync.dma_start(out=woT, in_=wo_d)
        nc.sync.dma_start(out=XTv[:, 2], in_=xt_d[256:384, :])
        nc.sync.dma_start(out=XTv[:, 3], in_=xt_d[384:512, :])

        # ---- stage emitters ----
        psq_live = {}

        def emit_qkv_chunk(ni, mi, c):
            # 4 of the 16 contraction tiles of the [128f, 512s] projection
            if c == 0:
                psq_live[(ni, mi)] = ps_qkv.tile([128, 512], f32, tag="ps_qkv",
                                                 name="psq")
            psq = psq_live[(ni, mi)]
            for dt in range(4 * c, 4 * c + 4):
                nc.tensor.matmul(
                    psq,
                    lhsT=wTv[:, dt, mi * 128:(mi + 1) * 128],
                    rhs=XTv[:, ni, dt, :],
                    start=(dt == 0),
                    stop=(dt == NT - 1),
                )
            if c == 3:
                del psq_live[(ni, mi)]
                nc.vector.tensor_scalar_add(
                    qkvT[mi][:, ni * 512:(ni + 1) * 512], psq, btl[mi])
                if mi == 2:
                    nc.sync.dma_start(
                        out=kdup[64:128, ni * 512:(ni + 1) * 512],
                        in_=qkvT[2][0:DK, ni * 512:(ni + 1) * 512],
                    )

        def emit_vt(ni):
            # V strips of this i-tile transposed into vS via PE
            psv = ps_f.tile([128, 4 * DK], bf16, tag="ps_f", name="psv")
            for j in range(4):
                nc.tensor.transpose(
                    psv[:, j * DK:(j + 1) * DK],
                    qkvT[2][64:128, (4 * ni + j) * 128:(4 * ni + j + 1) * 128],
                    ident_bf[64:128, 64:128],
                )
            nc.vector.tensor_copy(
                vSv[:, 4 * ni:4 * ni + 4, :],
                psv.rearrange("p (a b) -> p a b", a=4),
            )

        def emit_f(sb, mi2):
            psy = ps_f.tile([128, 512], f32, tag="ps_f", name="psy")
            for ft in range(2):
                nc.tensor.matmul(
                    psy,
                    lhsT=outT[ft][:, sb * 128:(sb + 1) * 128],
                    rhs=woTv[:, ft, mi2 * 512:(mi2 + 1) * 512],
                    start=(ft == 0),
                    stop=(ft == 1),
                )
            ysb = ysbp.tile([128, 512], f32, tag="ysbp", name="ysb")
            nc.vector.tensor_copy(ysb, psy)
            nc.sync.dma_start(
                out=y_d[sb * 128:(sb + 1) * 128, mi2 * 512:(mi2 + 1) * 512],
                in_=ysb,
            )

        def emit_scores(ti, p, bj):
            # head pair p: g_even at rows 0-63 (K source qkvT[2]), g_odd at
            # rows 64-127 (kdup). The two matmuls auto-derive tile_position
            # (0,0)/(64,0) -> they run concurrently in separate PE row groups.
            # Strips are computed full-width; causally dead columns are zeroed
            # after exp (keeps every PSUM byte initialized for the single exp).
            k = bj - 4 * ti
            sc = ps_sc.tile([128, 1024], f32, tag="ps_sc", name="sc")
            nc.tensor.matmul(
                sc[:, 0:512],
                lhsT=qkvT[2][0:DK, bj * 128:(bj + 1) * 128],
                rhs=qkvT[p][0:DK, ti * 512:(ti + 1) * 512],
                start=True, stop=True,
            )
            nc.tensor.matmul(
                sc[:, 512:1024],
                lhsT=kdup[64:128, bj * 128:(bj + 1) * 128],
                rhs=qkvT[p][64:128, ti * 512:(ti + 1) * 512],
                start=True, stop=True,
            )
            expT = expp.tile([128, 1024], bf16, tag="expp", name="expT")
            nc.scalar.activation(expT, sc, Exp, scale=0.125)
            if k >= 0:
                # keep where col >= row + 128k (zero the causally dead region)
                mw = min(512, 128 * k + 128)
                for lo in (0, 512):
                    nc.gpsimd.affine_select(
                        out=expT[:, lo:lo + mw],
                        in_=expT[:, lo:lo + mw],
                        compare_op=mybir.AluOpType.is_ge,
                        fill=0.0, base=-128 * k,
                        pattern=[[1, mw]], channel_multiplier=-1,
                    )
            return (expT,)

        def emit_avdn(bj, nstr, expT, aug, dn):
            first, last = bj == 0, bj == nstr - 1
            # col-tiled AV pair: g_even -> psum partitions 0-63, g_odd -> 64-127
            nc.tensor.matmul(
                aug[0:DK, :], lhsT=vSv[:, bj, :], rhs=expT[:, 0:512],
                start=first, stop=last, skip_group_check=True,
            )
            nc.tensor.matmul(
                aug[DK:128, :], lhsT=vSv[:, bj, :], rhs=expT[:, 512:1024],
                start=first, stop=last, skip_group_check=True,
            )
            # denominators: ones-column matmuls -> partitions 0 / 32 (concurrent)
            nc.tensor.matmul(
                dn[0:1, :], lhsT=onescol, rhs=expT[:, 0:512],
                start=first, stop=last, skip_group_check=True,
            )
            nc.tensor.matmul(
                dn[32:33, :], lhsT=onescol, rhs=expT[:, 512:1024],
                start=first, stop=last, skip_group_check=True,
            )

        def emit_norm(ti, p, aug, dn):
            if debug_dumps and ti == 3 and p == 1:
                daug = const.tile([128, 512], f32, name="daug")
                nc.vector.tensor_copy(daug, aug)
                nc.sync.dma_start(out=dbg["d_aug"], in_=daug)
                ddn = const.tile([33, 512], f32, name="ddn")
                nc.vector.tensor_copy(ddn[0:1, :], dn[0:1, :])
                nc.vector.tensor_copy(ddn[32:33, :], dn[32:33, :])
                nc.sync.dma_start(out=dbg["d_dn"], in_=ddn)
            rcp = rcpp.tile([33, 512], f32, tag="rcp", name="rcp")
            rcpB = rcpp.tile([33, 512], bf16, tag="rcpB", name="rcpB")
            # full-tile op: reciprocal_approx_fast mis-executes on HW for APs
            # with base partition != 0 (probed), so cover rows 0..32 in one op
            # (rows 1-31 are junk-in/junk-out, initialized once below)
            nc.vector.reciprocal_approx_fast(rcp, dn[0:33, :])
            nc.vector.tensor_copy(rcpB, rcp)
            # broadcast recips over the pair's rows, reusing dn's bank
            nc.tensor.matmul(dn[0:DK, :], lhsT=onesB[0:1, :], rhs=rcpB[0:1, :],
                             start=True, stop=True, skip_group_check=True)
            nc.tensor.matmul(dn[DK:128, :], lhsT=onesB[32:33, :],
                             rhs=rcpB[32:33, :], start=True, stop=True,
                             skip_group_check=True)
            # DVE may read only one PSUM operand: stage bc in SBUF via ACT
            bcS = rcpp.tile([128, 512], bf16, tag="bcS", name="bcS")
            nc.scalar.copy(bcS, dn)
            if debug_dumps and ti == 3 and p == 1:
                dbcS = const.tile([128, 512], f32, name="dbcS")
                nc.vector.tensor_copy(dbcS, bcS)
                nc.sync.dma_start(out=dbg["d_bcS"], in_=dbcS)
            nc.vector.tensor_mul(outT[p][:, ti * 512:(ti + 1) * 512], aug, bcS)

        # ---- pipelined schedule ----
        fill = deque()

        def pump(n):
            for _ in range(n):
                if not fill:
                    return
                fill.popleft()()

        for mi in range(3):
            for c in range(4):
                emit_qkv_chunk(0, mi, c)
        emit_vt(0)

        for ti in range(NI):
            if ti < NI - 1:
                for mi in range(3):
                    for c in range(4):
                        fill.append(lambda ni=ti + 1, mi=mi, c=c:
                                    emit_qkv_chunk(ni, mi, c))
                fill.append(lambda ni=ti + 1: emit_vt(ni))
            if ti >= 1:
                for sb in range(4 * (ti - 1), 4 * ti):
                    for mi2 in range(4):
                        fill.append(lambda sb=sb, mi2=mi2: emit_f(sb, mi2))
            nstr = 4 * ti + 4
            for p in range(2):
                aug = ps_av.tile([128, 512], f32, tag="ps_av", name="aug")
                dn = ps_dn.tile([128, 512], f32, tag="ps_dn", name="dn")
                if ti == 0 and p == 0:
                    # keep the junk rows of the shared dn bank finite/nonzero
                    # for the full-tile reciprocal (and sim's uninit tracking)
                    nc.vector.memset(dn[0:32, :], 1.0)
                prev = None
                for bj in range(nstr):
                    cur = emit_scores(ti, p, bj)
                    pump(1)
                    if prev is not None:
                        emit_avdn(bj - 1, nstr, *prev, aug, dn)
                    prev = cur
                pump(1)
                emit_avdn(nstr - 1, nstr, *prev, aug, dn)
                emit_norm(ti, p, aug, dn)
            pump(len(fill))
        for sb in range(4 * (NI - 1), 4 * NI):
            for mi2 in range(4):
                emit_f(sb, mi2)

        if debug_dumps:
            for nm, src in [("d_qkvT0", qkvT[0]), ("d_qkvT1", qkvT[1]),
                            ("d_qkvT2", qkvT[2]), ("d_kdup", kdup),
                            ("d_vS", vS), ("d_outT0", outT[0]),
                            ("d_outT1", outT[1])]:
                nc.sync.dma_start(out=dbg[nm], in_=src)

    nc.compile()
    return nc


def _get_nc():
    if "nc" not in _cache:
        _cache["nc"] = _build()
    return _cache["nc"]


def _in_maps(x, wq, bq, wk, bk, wv, bv, wo):
    import ml_dtypes

    bf = ml_dtypes.bfloat16
    x = np.asarray(x, np.float32)
    # xt[ni*128+p, dt*512+s'] = x[ni*512+s', dt*128+p]  (shared by all cores)
    xT = np.ascontiguousarray(x.T).astype(bf)                    # [d, s]
    xt = np.ascontiguousarray(
        xT.reshape(NT, 128, NI, 512).transpose(2, 1, 0, 3)
    ).reshape(NI * 128, NT * 512)
    maps = []
    for h in range(N_CORES):
        qs = slice(h * QF, (h + 1) * QF)
        ks = slice(h * DK, (h + 1) * DK)
        wqkv = np.concatenate([wq[qs], wk[ks], wv[ks]], axis=0)  # [384, 2048]
        wqkvT = np.ascontiguousarray(wqkv.T.astype(np.float32)).astype(bf)
        wt = np.ascontiguousarray(
            wqkvT.reshape(NT, 128, NF).transpose(1, 0, 2)
        ).reshape(128, NT * NF)
        woT = np.ascontiguousarray(wo[:, qs].T.astype(np.float32)).astype(bf)
        wob = np.ascontiguousarray(
            woT.reshape(2, 128, D).transpose(1, 0, 2)
        ).reshape(128, 2 * D)
        bqkv = np.concatenate([bq[qs], bk[ks], bv[ks]], axis=0).astype(np.float32)
        maps.append({
            "xt": xt,
            "wt": wt,
            "wob": wob,
            "bq": np.ascontiguousarray(bqkv[:, None]),
        })
    return maps


def _run(inputs, trace=False, tmpdir=None):
    from concourse.bass_utils import run_bass_kernel_spmd

    nc = _get_nc()
    x = np.asarray(inputs["x"])[0]
    maps = _in_maps(
        x,
        np.asarray(inputs["wq"]), np.asarray(inputs["bq"]),
        np.asarray(inputs["wk"]), np.asarray(inputs["bk"]),
        np.asarray(inputs["wv"]), np.asarray(inputs["bv"]),
        np.asarray(inputs["wo"]),
    )
    res = run_bass_kernel_spmd(
        nc, maps, list(range(N_CORES)), trace=trace, tmpdir=tmpdir
    )
    y = np.zeros((S, D), dtype=np.float32)
    for i in range(N_CORES):
        y += res.results[i]["y"]
    y += np.asarray(inputs["bo"])[None, :]
    return y[None], res


def kernel(**inputs):
    y, _ = _run(inputs, trace=False)
    return y.astype(np.float32)


# revision 13
# speedup vs baseline: 1.4043x; 1.0165x over previous
"""GQA kernel for Trainium2, 8-way tensor-parallel over kv heads.

Problem (hardcoded): B=1, S=2048, D_MODEL=2048, HQ=32, HKV=8, DK=64, causal.
Sharding: core h owns kv head h and its 4 query heads. Weights are sliced,
transposed and cast to bf16 on host; x is replicated (transposed bf16). Each
core computes a partial y (its heads projected through its slice of wo); host
sums the 8 partials and adds bo.

On-chip dataflow per core (all matmuls bf16 with fp32 PSUM accumulation):
  xT, wqkvT, woT arrive pre-transposed -> no PE transposes for x/weights
  qkvT[f, s] projection, evacuated via DVE with fused per-partition bias
  scores: heads processed in pairs (g_even at array rows 0-63, g_odd at rows
    64-127 via duplicated K) -> the two 64-contraction matmuls run
    CONCURRENTLY in separate PE row groups (tile_position auto-derived)
  expT = exp(0.125 * scoresT) on ACT, causal strips only; diagonal blocks
    masked by gpsimd affine_select on the leading 128 columns only
  AV: col-tiled pair -- head g_even -> PSUM partitions 0-63, g_odd -> 64-127
    of one bank (concurrent), V is 64-wide; denominators via ones-column
    matmuls into partitions 0 / 32 of a second bank (concurrent pair)
  normalize: reciprocal_approx_fast + tiny broadcast matmuls + one DVE mul
  y = outT.T @ woT per 128-row block, interleaved into attention slack

Emission is software-pipelined with a filler queue: F(ti-1) and QKV(ti+1)
matmul chunks are popped between attention strips so PE stays busy while ACT
computes exp (ACT paces the attention phase).
"""

from collections import deque

import numpy as np

S = 2048
D = 2048
HQ, HKV, GRP, DK = 32, 8, 4, 64
QF = GRP * DK            # 256 query features per core
NF = QF + 2 * DK         # 384 projected features per core
N_CORES = 8
NT = S // 128            # 16 d-tiles
NI = S // 512            # 4 i-tiles

_cache = {}


def _build(debug_dumps=False):
    import concourse.bass as bass
    import concourse.mybir as mybir
    from concourse import bacc, tile
    from concourse.masks import make_identity
    from contextlib import ExitStack

    f32 = mybir.dt.float32
    bf16 = mybir.dt.bfloat16
    Exp = mybir.ActivationFunctionType.Exp

    nc = bacc.Bacc(
        "TRN2",
        target_bir_lowering=False,
        debug=False,
        enable_asserts=False,
        num_devices=N_CORES,
    )
    # host layouts (see _in_maps): xt[ni*128+p, dt*512+s'] = x[ni*512+s', dt*128+p]
    xt_d = nc.dram_tensor("xt", [NI * 128, NT * 512], bf16, kind="ExternalInput").ap()
    wt_d = nc.dram_tensor("wt", [128, NT * NF], bf16, kind="ExternalInput").ap()
    wo_d = nc.dram_tensor("wob", [128, 2 * D], bf16, kind="ExternalInput").ap()
    bq_d = nc.dram_tensor("bq", [NF, 1], f32, kind="ExternalInput").ap()
    y_d = nc.dram_tensor("y", [S, D], bf16, kind="ExternalOutput").ap()
    dbg = {}
    if debug_dumps:
        for nm, shp in [("d_qkvT0", [128, S]), ("d_qkvT1", [128, S]),
                        ("d_qkvT2", [128, S]), ("d_kdup", [128, S]),
                        ("d_vS", [128, NT * DK]), ("d_outT0", [128, S]),
                        ("d_outT1", [128, S])]:
            dbg[nm] = nc.dram_tensor(nm, shp, bf16, kind="ExternalOutput").ap()
        for nm, shp in [("d_aug", [128, 512]), ("d_dn", [33, 512]),
                        ("d_bcS", [128, 512])]:
            dbg[nm] = nc.dram_tensor(nm, shp, f32, kind="ExternalOutput").ap()

    with tile.TileContext(nc) as tc, ExitStack() as ctx:
        const = ctx.enter_context(tc.tile_pool(name="const", bufs=1))
        expp = ctx.enter_context(tc.tile_pool(name="expp", bufs=4))
        ysbp = ctx.enter_context(tc.tile_pool(name="ysbp", bufs=4))
        rcpp = ctx.enter_context(tc.tile_pool(name="rcpp", bufs=2))
        # PSUM (8 banks): sc 2x[128,1024]=4, av 1, dn/bc 1, qkv 1, f 1
        ps_sc = ctx.enter_context(tc.tile_pool(name="ps_sc", bufs=2, space="PSUM"))
        ps_av = ctx.enter_context(tc.tile_pool(name="ps_av", bufs=1, space="PSUM"))
        ps_dn = ctx.enter_context(tc.tile_pool(name="ps_dn", bufs=1, space="PSUM"))
        ps_qkv = ctx.enter_context(tc.tile_pool(name="ps_qkv", bufs=1, space="PSUM"))
        ps_f = ctx.enter_context(tc.tile_pool(name="ps_f", bufs=1, space="PSUM"))

        # ---- constants ----
        ident_bf = const.tile([128, 128], bf16)
        make_identity(nc, ident_bf)
        onescol = const.tile([128, 1], bf16)
        nc.gpsimd.memset(onescol, 1.0)
        onesB = const.tile([33, 64], bf16)
        nc.gpsimd.memset(onesB, 1.0)

        # ---- persistent SBUF ----
        XT = const.tile([128, NI * NT * 512], bf16)
        XTv = XT.rearrange("p (n t s) -> p n t s", n=NI, t=NT)
        wT = const.tile([128, NT * NF], bf16)
        wTv = wT.rearrange("p (t f) -> p t f", t=NT)
        woT = const.tile([128, 2 * D], bf16)
        woTv = woT.rearrange("p (t m) -> p t m", t=2)
        # qkvT[0] = Q heads g0|g1 (rows 0-63|64-127), [1] = g2|g3, [2] = K|V
        qkvT = [const.tile([128, S], bf16, name=f"qkvT{i}") for i in range(3)]
        kdup = const.tile([128, S], bf16)           # K duplicated at rows 64-127
        vS = const.tile([128, NT * DK], bf16)       # V as [s%128, strip, f]
        vSv = vS.rearrange("p (j f) -> p j f", j=NT)
        outT = [const.tile([128, S], bf16, name=f"outT{i}") for i in range(2)]
        btl = [const.tile([128, 1], f32, name=f"btl{i}") for i in range(3)]

        # ---- upfront DMAs (x slabs ordered by first use) ----
        nc.sync.dma_start(out=XTv[:, 0], in_=xt_d[0:128, :])
        nc.sync.dma_start(out=wT, in_=wt_d)
        for mi in range(3):
            nc.sync.dma_start(out=btl[mi], in_=bq_d[mi * 128:(mi + 1) * 128, :])
        nc.sync.dma_start(out=XTv[:, 1], in_=xt_d[128:256, :])
        nc.sync.dma_start(out=woT, in_=wo_d)
        nc.sync.dma_start(out=XTv[:, 2], in_=xt_d[256:384, :])
        nc.sync.dma_start(out=XTv[:, 3], in_=xt_d[384:512, :])

        # ---- stage emitters ----
        psq_live = {}

        def emit_qkv_chunk(nis, mi, c):
            # 4 of the 16 contraction tiles of the [128f, 512s] projection;
            # paired nis share the stationary weights (walrus dedupes the
            # LDWEIGHTS of consecutive same-lhsT matmuls)
            if c == 0:
                for ni in nis:
                    psq_live[(ni, mi)] = ps_qkv.tile(
                        [128, 512], f32, tag="ps_qkv", name="psq")
            for dt in range(4 * c, 4 * c + 4):
                for ni in nis:
                    nc.tensor.matmul(
                        psq_live[(ni, mi)],
                        lhsT=wTv[:, dt, mi * 128:(mi + 1) * 128],
                        rhs=XTv[:, ni, dt, :],
                        start=(dt == 0),
                        stop=(dt == NT - 1),
                    )
            if c == 3:
                for ni in nis:
                    psq = psq_live.pop((ni, mi))
                    nc.vector.tensor_scalar_add(
                        qkvT[mi][:, ni * 512:(ni + 1) * 512], psq, btl[mi])
                    if mi == 2:
                        nc.sync.dma_start(
                            out=kdup[64:128, ni * 512:(ni + 1) * 512],
                            in_=qkvT[2][0:DK, ni * 512:(ni + 1) * 512],
                        )

        def emit_vt(ni):
            # V strips of this i-tile transposed into vS via PE
            psv = ps_f.tile([128, 4 * DK], bf16, tag="ps_f", name="psv")
            for j in range(4):
                nc.tensor.transpose(
                    psv[:, j * DK:(j + 1) * DK],
                    qkvT[2][64:128, (4 * ni + j) * 128:(4 * ni + j + 1) * 128],
                    ident_bf[64:128, 64:128],
                )
            nc.vector.tensor_copy(
                vSv[:, 4 * ni:4 * ni + 4, :],
                psv.rearrange("p (a b) -> p a b", a=4),
            )

        def emit_f(sb, mi2):
            psy = ps_f.tile([128, 512], f32, tag="ps_f", name="psy")
            for ft in range(2):
                nc.tensor.matmul(
                    psy,
                    lhsT=outT[ft][:, sb * 128:(sb + 1) * 128],
                    rhs=woTv[:, ft, mi2 * 512:(mi2 + 1) * 512],
                    start=(ft == 0),
                    stop=(ft == 1),
                )
            ysb = ysbp.tile([128, 512], bf16, tag="ysbp", name="ysb")
            nc.vector.tensor_copy(ysb, psy)
            nc.sync.dma_start(
                out=y_d[sb * 128:(sb + 1) * 128, mi2 * 512:(mi2 + 1) * 512],
                in_=ysb,
            )

        def emit_scores(ti, p, bj):
            # head pair p: g_even at rows 0-63 (K source qkvT[2]), g_odd at
            # rows 64-127 (kdup). The two matmuls auto-derive tile_position
            # (0,0)/(64,0) -> they run concurrently in separate PE row groups.
            # Strips are computed full-width; causally dead columns are zeroed
            # after exp (keeps every PSUM byte initialized for the single exp).
            k = bj - 4 * ti
            sc = ps_sc.tile([128, 1024], f32, tag="ps_sc", name="sc")
            nc.tensor.matmul(
                sc[:, 0:512],
                lhsT=qkvT[2][0:DK, bj * 128:(bj + 1) * 128],
                rhs=qkvT[p][0:DK, ti * 512:(ti + 1) * 512],
                start=True, stop=True,
            )
            nc.tensor.matmul(
                sc[:, 512:1024],
                lhsT=kdup[64:128, bj * 128:(bj + 1) * 128],
                rhs=qkvT[p][64:128, ti * 512:(ti + 1) * 512],
                start=True, stop=True,
            )
            expT = expp.tile([128, 1024], bf16, tag="expp", name="expT")
            nc.scalar.activation(expT, sc, Exp, scale=0.125)
            if k >= 0:
                # keep where col >= row + 128k (zero the causally dead region)
                mw = min(512, 128 * k + 128)
                for lo in (0, 512):
                    nc.gpsimd.affine_select(
                        out=expT[:, lo:lo + mw],
                        in_=expT[:, lo:lo + mw],
                        compare_op=mybir.AluOpType.is_ge,
                        fill=0.0, base=-128 * k,
                        pattern=[[1, mw]], channel_multiplier=-1,
                    )
            return (expT,)

        def emit_avdn(bj, nstr, expT, aug, dn):
            first, last = bj == 0, bj == nstr - 1
            # col-tiled AV pair: g_even -> psum partitions 0-63, g_odd -> 64-127
            nc.tensor.matmul(
                aug[0:DK, :], lhsT=vSv[:, bj, :], rhs=expT[:, 0:512],
                start=first, stop=last, skip_group_check=True,
            )
            nc.tensor.matmul(
                aug[DK:128, :], lhsT=vSv[:, bj, :], rhs=expT[:, 512:1024],
                start=first, stop=last, skip_group_check=True,
            )
            # denominators: ones-column matmuls -> partitions 0 / 32 (concurrent)
            nc.tensor.matmul(
                dn[0:1, :], lhsT=onescol, rhs=expT[:, 0:512],
                start=first, stop=last, skip_group_check=True,
            )
            nc.tensor.matmul(
                dn[32:33, :], lhsT=onescol, rhs=expT[:, 512:1024],
                start=first, stop=last, skip_group_check=True,
            )

        def emit_norm(ti, p, aug, dn):
            if debug_dumps and ti == 3 and p == 1:
                daug = const.tile([128, 512], f32, name="daug")
                nc.vector.tensor_copy(daug, aug)
                nc.sync.dma_start(out=dbg["d_aug"], in_=daug)
                ddn = const.tile([33, 512], f32, name="ddn")
                nc.vector.tensor_copy(ddn[0:1, :], dn[0:1, :])
                nc.vector.tensor_copy(ddn[32:33, :], dn[32:33, :])
                nc.sync.dma_start(out=dbg["d_dn"], in_=ddn)
            rcp = rcpp.tile([33, 512], f32, tag="rcp", name="rcp")
            rcpB = rcpp.tile([33, 512], bf16, tag="rcpB", name="rcpB")
            # full-tile op: reciprocal_approx_fast mis-executes on HW for APs
            # with base partition != 0 (probed), so cover rows 0..32 in one op
            # (rows 1-31 are junk-in/junk-out, initialized once below)
            nc.vector.reciprocal_approx_fast(rcp, dn[0:33, :])
            nc.vector.tensor_copy(rcpB, rcp)
            # broadcast recips over the pair's rows, reusing dn's bank
            nc.tensor.matmul(dn[0:DK, :], lhsT=onesB[0:1, :], rhs=rcpB[0:1, :],
                             start=True, stop=True, skip_group_check=True)
            nc.tensor.matmul(dn[DK:128, :], lhsT=onesB[32:33, :],
                             rhs=rcpB[32:33, :], start=True, stop=True,
                             skip_group_check=True)
            # DVE may read only one PSUM operand: stage bc in SBUF via ACT
            bcS = rcpp.tile([128, 512], bf16, tag="bcS", name="bcS")
            nc.scalar.copy(bcS, dn)
            if debug_dumps and ti == 3 and p == 1:
                dbcS = const.tile([128, 512], f32, name="dbcS")
                nc.vector.tensor_copy(dbcS, bcS)
                nc.sync.dma_start(out=dbg["d_bcS"], in_=dbcS)
            nc.vector.tensor_mul(outT[p][:, ti * 512:(ti + 1) * 512], aug, bcS)

        # ---- pipelined schedule ----
        fill = deque()

        def pump(n):
            for _ in range(n):
                if not fill:
                    return
                fill.popleft()()

        for mi in range(3):
            for c in range(4):
                emit_qkv_chunk((0,), mi, c)
        emit_vt(0)

        for ti in range(NI):
            if ti < NI - 1:
                for mi in range(3):
                    for c in range(4):
                        fill.append(lambda ni=ti + 1, mi=mi, c=c:
                                    emit_qkv_chunk((ni,), mi, c))
                fill.append(lambda ni=ti + 1: emit_vt(ni))
            if ti >= 1:
                for sb in range(4 * (ti - 1), 4 * ti):
                    for mi2 in range(4):
                        fill.append(lambda sb=sb, mi2=mi2: emit_f(sb, mi2))
            nstr = 4 * ti + 4
            for p in range(2):
                aug = ps_av.tile([128, 512], f32, tag="ps_av", name="aug")
                dn = ps_dn.tile([128, 512], f32, tag="ps_dn", name="dn")
                if ti == 0 and p == 0:
                    # keep the junk rows of the shared dn bank finite/nonzero
                    # for the full-tile reciprocal (and sim's uninit tracking)
                    nc.vector.memset(dn[0:32, :], 1.0)
                prev = None
                for bj in range(nstr):
                    cur = emit_scores(ti, p, bj)
                    pump(1)
                    if prev is not None:
                        emit_avdn(bj - 1, nstr, *prev, aug, dn)
                    prev = cur
                pump(1)
                emit_avdn(nstr - 1, nstr, *prev, aug, dn)
                emit_norm(ti, p, aug, dn)
            pump(len(fill))
        for sb in range(4 * (NI - 1), 4 * NI):
            for mi2 in range(4):
                emit_f(sb, mi2)

        if debug_dumps:
            for nm, src in [("d_qkvT0", qkvT[0]), ("d_qkvT1", qkvT[1]),
                            ("d_qkvT2", qkvT[2]), ("d_kdup", kdup),
                            ("d_vS", vS), ("d_outT0", outT[0]),
                            ("d_outT1", outT[1])]:
                nc.sync.dma_start(out=dbg[nm], in_=src)

    nc.compile()
    return nc


def _get_nc():
    if "nc" not in _cache:
        _cache["nc"] = _build()
    return _cache["nc"]


def _in_maps(x, wq, bq, wk, bk, wv, bv, wo):
    import ml_dtypes

    bf = ml_dtypes.bfloat16
    x = np.asarray(x, np.float32)
    # xt[ni*128+p, dt*512+s'] = x[ni*512+s', dt*128+p]  (shared by all cores)
    xT = np.ascontiguousarray(x.T).astype(bf)                    # [d, s]
    xt = np.ascontiguousarray(
        xT.reshape(NT, 128, NI, 512).transpose(2, 1, 0, 3)
    ).reshape(NI * 128, NT * 512)
    maps = []
    for h in range(N_CORES):
        qs = slice(h * QF, (h + 1) * QF)
        ks = slice(h * DK, (h + 1) * DK)
        wqkv = np.concatenate([wq[qs], wk[ks], wv[ks]], axis=0)  # [384, 2048]
        wqkvT = np.ascontiguousarray(wqkv.T.astype(np.float32)).astype(bf)
        wt = np.ascontiguousarray(
            wqkvT.reshape(NT, 128, NF).transpose(1, 0, 2)
        ).reshape(128, NT * NF)
        woT = np.ascontiguousarray(wo[:, qs].T.astype(np.float32)).astype(bf)
        wob = np.ascontiguousarray(
            woT.reshape(2, 128, D).transpose(1, 0, 2)
        ).reshape(128, 2 * D)
        bqkv = np.concatenate([bq[qs], bk[ks], bv[ks]], axis=0).astype(np.float32)
        maps.append({
            "xt": xt,
            "wt": wt,
            "wob": wob,
            "bq": np.ascontiguousarray(bqkv[:, None]),
        })
    return maps


def _run(inputs, trace=False, tmpdir=None):
    from concourse.bass_utils import run_bass_kernel_spmd

    nc = _get_nc()
    x = np.asarray(inputs["x"])[0]
    maps = _in_maps(
        x,
        np.asarray(inputs["wq"]), np.asarray(inputs["bq"]),
        np.asarray(inputs["wk"]), np.asarray(inputs["bk"]),
        np.asarray(inputs["wv"]), np.asarray(inputs["bv"]),
        np.asarray(inputs["wo"]),
    )
    res = run_bass_kernel_spmd(
        nc, maps, list(range(N_CORES)), trace=trace, tmpdir=tmpdir
    )
    y = np.zeros((S, D), dtype=np.float32)
    for i in range(N_CORES):
        y += np.asarray(res.results[i]["y"]).astype(np.float32)
    y += np.asarray(inputs["bo"])[None, :]
    return y[None], res


def kernel(**inputs):
    y, _ = _run(inputs, trace=False)
    return y.astype(np.float32)


# revision 14
# speedup vs baseline: 1.4929x; 1.0631x over previous
"""GQA kernel for Trainium2, 8-way tensor-parallel over kv heads.

Problem (hardcoded): B=1, S=2048, D_MODEL=2048, HQ=32, HKV=8, DK=64, causal.
Sharding: core h owns kv head h and its 4 query heads. Weights are sliced,
transposed and cast to bf16 on host; x is replicated (transposed bf16). Each
core computes a partial y (its heads projected through its slice of wo); host
sums the 8 partials and adds bo.

On-chip dataflow per core (all matmuls bf16 with fp32 PSUM accumulation):
  xT, wqkvT, woT arrive pre-transposed -> no PE transposes for x/weights
  qkvT[f, s] projection, evacuated via DVE with fused per-partition bias
  scores: heads processed in pairs (g_even at array rows 0-63, g_odd at rows
    64-127 via duplicated K) -> the two 64-contraction matmuls run
    CONCURRENTLY in separate PE row groups (tile_position auto-derived)
  expT = exp(0.125 * scoresT) on ACT, causal strips only; diagonal blocks
    masked by gpsimd affine_select on the leading 128 columns only
  AV: col-tiled pair -- head g_even -> PSUM partitions 0-63, g_odd -> 64-127
    of one bank (concurrent), V is 64-wide; denominators via ones-column
    matmuls into partitions 0 / 32 of a second bank (concurrent pair)
  normalize: reciprocal_approx_fast + tiny broadcast matmuls + one DVE mul
  y = outT.T @ woT per 128-row block, interleaved into attention slack

Emission is software-pipelined with a filler queue: F(ti-1) and QKV(ti+1)
matmul chunks are popped between attention strips so PE stays busy while ACT
computes exp (ACT paces the attention phase).
"""

from collections import deque

import numpy as np

S = 2048
D = 2048
HQ, HKV, GRP, DK = 32, 8, 4, 64
QF = GRP * DK            # 256 query features per core
NF = QF + 2 * DK         # 384 projected features per core
N_CORES = 8
NT = S // 128            # 16 d-tiles
NI = S // 512            # 4 i-tiles

_cache = {}


def _build(debug_dumps=False):
    import concourse.bass as bass
    import concourse.mybir as mybir
    from concourse import bacc, tile
    from concourse.masks import make_identity
    from contextlib import ExitStack

    f32 = mybir.dt.float32
    bf16 = mybir.dt.bfloat16
    Exp = mybir.ActivationFunctionType.Exp

    nc = bacc.Bacc(
        "TRN2",
        target_bir_lowering=False,
        debug=False,
        enable_asserts=False,
        num_devices=N_CORES,
    )
    # host layouts (see _in_maps): xt[ni*128+p, dt*512+s'] = x[ni*512+s', dt*128+p]
    xt_d = nc.dram_tensor("xt", [NI * 128, NT * 512], bf16, kind="ExternalInput").ap()
    wt_d = nc.dram_tensor("wt", [128, NT * NF], bf16, kind="ExternalInput").ap()
    wo_d = nc.dram_tensor("wob", [128, 2 * D], bf16, kind="ExternalInput").ap()
    bq_d = nc.dram_tensor("bq", [NF, 1], f32, kind="ExternalInput").ap()
    y_d = nc.dram_tensor("y", [S, D], bf16, kind="ExternalOutput").ap()
    dbg = {}
    if debug_dumps:
        for nm, shp in [("d_qkvT0", [128, S]), ("d_qkvT1", [128, S]),
                        ("d_qkvT2", [128, S]), ("d_kdup", [128, S]),
                        ("d_vS", [128, NT * DK]), ("d_outT0", [128, S]),
                        ("d_outT1", [128, S])]:
            dbg[nm] = nc.dram_tensor(nm, shp, bf16, kind="ExternalOutput").ap()
        for nm, shp in [("d_aug", [128, 512]), ("d_dn", [33, 512]),
                        ("d_bcS", [128, 512])]:
            dbg[nm] = nc.dram_tensor(nm, shp, f32, kind="ExternalOutput").ap()

    with tile.TileContext(nc) as tc, ExitStack() as ctx:
        const = ctx.enter_context(tc.tile_pool(name="const", bufs=1))
        expp = ctx.enter_context(tc.tile_pool(name="expp", bufs=4))
        ysbp = ctx.enter_context(tc.tile_pool(name="ysbp", bufs=4))
        rcpp = ctx.enter_context(tc.tile_pool(name="rcpp", bufs=2))
        # PSUM (8 banks): sc 2x[128,1024]=4, av 1, dn/bc 1, qkv 1, f 1
        ps_sc = ctx.enter_context(tc.tile_pool(name="ps_sc", bufs=2, space="PSUM"))
        ps_av = ctx.enter_context(tc.tile_pool(name="ps_av", bufs=1, space="PSUM"))
        ps_dn = ctx.enter_context(tc.tile_pool(name="ps_dn", bufs=1, space="PSUM"))
        ps_qkv = ctx.enter_context(tc.tile_pool(name="ps_qkv", bufs=1, space="PSUM"))
        ps_f = ctx.enter_context(tc.tile_pool(name="ps_f", bufs=1, space="PSUM"))

        # ---- constants ----
        ident_bf = const.tile([128, 128], bf16)
        make_identity(nc, ident_bf)
        onescol = const.tile([128, 1], bf16)
        nc.gpsimd.memset(onescol, 1.0)
        onesB = const.tile([33, 64], bf16)
        nc.gpsimd.memset(onesB, 1.0)

        # ---- persistent SBUF ----
        XT = const.tile([128, NI * NT * 512], bf16)
        XTv = XT.rearrange("p (n t s) -> p n t s", n=NI, t=NT)
        wT = const.tile([128, NT * NF], bf16)
        wTv = wT.rearrange("p (t f) -> p t f", t=NT)
        woT = const.tile([128, 2 * D], bf16)
        woTv = woT.rearrange("p (t m) -> p t m", t=2)
        # qkvT[0] = Q heads g0|g1 (rows 0-63|64-127), [1] = g2|g3, [2] = K|V
        qkvT = [const.tile([128, S], bf16, name=f"qkvT{i}") for i in range(3)]
        kdup = const.tile([128, S], bf16)           # K duplicated at rows 64-127
        vS = const.tile([128, NT * DK], bf16)       # V as [s%128, strip, f]
        vSv = vS.rearrange("p (j f) -> p j f", j=NT)
        outT = [const.tile([128, S], bf16, name=f"outT{i}") for i in range(2)]
        btl = [const.tile([128, 1], f32, name=f"btl{i}") for i in range(3)]

        # ---- upfront DMAs (x slabs ordered by first use) ----
        nc.sync.dma_start(out=XTv[:, 0], in_=xt_d[0:128, :])
        nc.sync.dma_start(out=wT, in_=wt_d)
        for mi in range(3):
            nc.sync.dma_start(out=btl[mi], in_=bq_d[mi * 128:(mi + 1) * 128, :])
        nc.sync.dma_start(out=XTv[:, 1], in_=xt_d[128:256, :])
        nc.sync.dma_start(out=woT, in_=wo_d)
        nc.sync.dma_start(out=XTv[:, 2], in_=xt_d[256:384, :])
        nc.sync.dma_start(out=XTv[:, 3], in_=xt_d[384:512, :])

        # ---- stage emitters ----
        psq_live = {}

        def emit_qkv_chunk(nis, mi, c):
            # 4 of the 16 contraction tiles of the [128f, 512s] projection;
            # paired nis share the stationary weights (walrus dedupes the
            # LDWEIGHTS of consecutive same-lhsT matmuls)
            if c == 0:
                for ni in nis:
                    psq_live[(ni, mi)] = ps_qkv.tile(
                        [128, 512], f32, tag="ps_qkv", name="psq")
            for dt in range(4 * c, 4 * c + 4):
                for ni in nis:
                    nc.tensor.matmul(
                        psq_live[(ni, mi)],
                        lhsT=wTv[:, dt, mi * 128:(mi + 1) * 128],
                        rhs=XTv[:, ni, dt, :],
                        start=(dt == 0),
                        stop=(dt == NT - 1),
                    )
            if c == 3:
                for ni in nis:
                    psq = psq_live.pop((ni, mi))
                    nc.vector.tensor_scalar_add(
                        qkvT[mi][:, ni * 512:(ni + 1) * 512], psq, btl[mi])
                    if mi == 2:
                        nc.sync.dma_start(
                            out=kdup[64:128, ni * 512:(ni + 1) * 512],
                            in_=qkvT[2][0:DK, ni * 512:(ni + 1) * 512],
                        )

        def emit_vt(ni):
            # V strips of this i-tile transposed into vS via PE
            psv = ps_f.tile([128, 4 * DK], bf16, tag="ps_f", name="psv")
            for j in range(4):
                nc.tensor.transpose(
                    psv[:, j * DK:(j + 1) * DK],
                    qkvT[2][64:128, (4 * ni + j) * 128:(4 * ni + j + 1) * 128],
                    ident_bf[64:128, 64:128],
                )
            nc.vector.tensor_copy(
                vSv[:, 4 * ni:4 * ni + 4, :],
                psv.rearrange("p (a b) -> p a b", a=4),
            )

        def emit_f(sb, mi2):
            psy = ps_f.tile([128, 512], f32, tag="ps_f", name="psy")
            for ft in range(2):
                nc.tensor.matmul(
                    psy,
                    lhsT=outT[ft][:, sb * 128:(sb + 1) * 128],
                    rhs=woTv[:, ft, mi2 * 512:(mi2 + 1) * 512],
                    start=(ft == 0),
                    stop=(ft == 1),
                )
            ysb = ysbp.tile([128, 512], bf16, tag="ysbp", name="ysb")
            nc.vector.tensor_copy(ysb, psy)
            nc.sync.dma_start(
                out=y_d[sb * 128:(sb + 1) * 128, mi2 * 512:(mi2 + 1) * 512],
                in_=ysb,
            )

        def emit_scores(ti, p, bj):
            # head pair p: g_even at rows 0-63 (K source qkvT[2]), g_odd at
            # rows 64-127 (kdup). The two matmuls auto-derive tile_position
            # (0,0)/(64,0) -> they run concurrently in separate PE row groups.
            # Strips are computed full-width; causally dead columns are zeroed
            # after exp (keeps every PSUM byte initialized for the single exp).
            k = bj - 4 * ti
            sc = ps_sc.tile([128, 1024], f32, tag="ps_sc", name="sc")
            nc.tensor.matmul(
                sc[:, 0:512],
                lhsT=qkvT[2][0:DK, bj * 128:(bj + 1) * 128],
                rhs=qkvT[p][0:DK, ti * 512:(ti + 1) * 512],
                start=True, stop=True,
            )
            nc.tensor.matmul(
                sc[:, 512:1024],
                lhsT=kdup[64:128, bj * 128:(bj + 1) * 128],
                rhs=qkvT[p][64:128, ti * 512:(ti + 1) * 512],
                start=True, stop=True,
            )
            expT = expp.tile([128, 1024], bf16, tag="expp", name="expT")
            nc.scalar.activation(expT, sc, Exp, scale=0.125)
            if k >= 0:
                # keep where col >= row + 128k (zero the causally dead region)
                mw = min(512, 128 * k + 128)
                for lo in (0, 512):
                    nc.gpsimd.affine_select(
                        out=expT[:, lo:lo + mw],
                        in_=expT[:, lo:lo + mw],
                        compare_op=mybir.AluOpType.is_ge,
                        fill=0.0, base=-128 * k,
                        pattern=[[1, mw]], channel_multiplier=-1,
                    )
            return (expT,)

        def emit_avdn(bj, nstr, expT, aug, dn):
            first, last = bj == 0, bj == nstr - 1
            # col-tiled AV pair: g_even -> psum partitions 0-63, g_odd -> 64-127
            nc.tensor.matmul(
                aug[0:DK, :], lhsT=vSv[:, bj, :], rhs=expT[:, 0:512],
                start=first, stop=last, skip_group_check=True,
            )
            nc.tensor.matmul(
                aug[DK:128, :], lhsT=vSv[:, bj, :], rhs=expT[:, 512:1024],
                start=first, stop=last, skip_group_check=True,
            )
            # denominators: ones-column matmuls -> partitions 0 / 32 (concurrent)
            nc.tensor.matmul(
                dn[0:1, :], lhsT=onescol, rhs=expT[:, 0:512],
                start=first, stop=last, skip_group_check=True,
            )
            nc.tensor.matmul(
                dn[32:33, :], lhsT=onescol, rhs=expT[:, 512:1024],
                start=first, stop=last, skip_group_check=True,
            )

        def emit_norm(ti, p, aug, dn):
            if debug_dumps and ti == 3 and p == 1:
                daug = const.tile([128, 512], f32, name="daug")
                nc.vector.tensor_copy(daug, aug)
                nc.sync.dma_start(out=dbg["d_aug"], in_=daug)
                ddn = const.tile([33, 512], f32, name="ddn")
                nc.vector.tensor_copy(ddn[0:1, :], dn[0:1, :])
                nc.vector.tensor_copy(ddn[32:33, :], dn[32:33, :])
                nc.sync.dma_start(out=dbg["d_dn"], in_=ddn)
            rcp = rcpp.tile([33, 512], f32, tag="rcp", name="rcp")
            rcpB = rcpp.tile([33, 512], bf16, tag="rcpB", name="rcpB")
            # full-tile op: reciprocal_approx_fast mis-executes on HW for APs
            # with base partition != 0 (probed), so cover rows 0..32 in one op
            # (rows 1-31 are junk-in/junk-out, initialized once below)
            nc.vector.reciprocal_approx_fast(rcp, dn[0:33, :])
            nc.vector.tensor_copy(rcpB, rcp)
            # broadcast recips over the pair's rows, reusing dn's bank
            nc.tensor.matmul(dn[0:DK, :], lhsT=onesB[0:1, :], rhs=rcpB[0:1, :],
                             start=True, stop=True, skip_group_check=True)
            nc.tensor.matmul(dn[DK:128, :], lhsT=onesB[32:33, :],
                             rhs=rcpB[32:33, :], start=True, stop=True,
                             skip_group_check=True)
            # DVE may read only one PSUM operand: stage bc in SBUF via ACT
            bcS = rcpp.tile([128, 512], bf16, tag="bcS", name="bcS")
            nc.scalar.copy(bcS, dn)
            if debug_dumps and ti == 3 and p == 1:
                dbcS = const.tile([128, 512], f32, name="dbcS")
                nc.vector.tensor_copy(dbcS, bcS)
                nc.sync.dma_start(out=dbg["d_bcS"], in_=dbcS)
            nc.vector.tensor_mul(outT[p][:, ti * 512:(ti + 1) * 512], aug, bcS)

        # ---- pipelined schedule ----
        fill = deque()

        def pump(n):
            for _ in range(n):
                if not fill:
                    return
                fill.popleft()()

        for mi in range(3):
            for c in range(4):
                emit_qkv_chunk((0,), mi, c)
        emit_vt(0)

        for ti in range(NI):
            if ti < NI - 1:
                for mi in range(3):
                    for c in range(4):
                        fill.append(lambda ni=ti + 1, mi=mi, c=c:
                                    emit_qkv_chunk((ni,), mi, c))
                fill.append(lambda ni=ti + 1: emit_vt(ni))
            if ti >= 1:
                for sb in range(4 * (ti - 1), 4 * ti):
                    for mi2 in range(4):
                        fill.append(lambda sb=sb, mi2=mi2: emit_f(sb, mi2))
            nstr = 4 * ti + 4
            for p in range(2):
                aug = ps_av.tile([128, 512], f32, tag="ps_av", name="aug")
                dn = ps_dn.tile([128, 512], f32, tag="ps_dn", name="dn")
                if ti == 0 and p == 0:
                    # keep the junk rows of the shared dn bank finite/nonzero
                    # for the full-tile reciprocal (and sim's uninit tracking)
                    nc.vector.memset(dn[0:32, :], 1.0)
                prev = None
                for bj in range(nstr):
                    cur = emit_scores(ti, p, bj)
                    pump(1)
                    if prev is not None:
                        emit_avdn(bj - 1, nstr, *prev, aug, dn)
                    prev = cur
                pump(1)
                emit_avdn(nstr - 1, nstr, *prev, aug, dn)
                emit_norm(ti, p, aug, dn)
            pump(len(fill))
        # tail: attention banks are idle now -- rotate the final F through the
        # freed ps_sc slots with alternating evac engines so the PE stays
        # dense (and HAM stays warm) to the end
        for i, (sb, mi2) in enumerate(
                (sb, mi2) for sb in range(4 * (NI - 1), 4 * NI)
                for mi2 in range(4)):
            psy = ps_sc.tile([128, 512], f32, tag="ps_sc", name="psyt")
            for ft in range(2):
                nc.tensor.matmul(
                    psy,
                    lhsT=outT[ft][:, sb * 128:(sb + 1) * 128],
                    rhs=woTv[:, ft, mi2 * 512:(mi2 + 1) * 512],
                    start=(ft == 0),
                    stop=(ft == 1),
                )
            ysb = ysbp.tile([128, 512], bf16, tag="ysbp", name="ysb")
            if i % 2 == 0:
                nc.vector.tensor_copy(ysb, psy)
            else:
                nc.scalar.copy(ysb, psy)
            nc.sync.dma_start(
                out=y_d[sb * 128:(sb + 1) * 128, mi2 * 512:(mi2 + 1) * 512],
                in_=ysb,
            )

        if debug_dumps:
            for nm, src in [("d_qkvT0", qkvT[0]), ("d_qkvT1", qkvT[1]),
                            ("d_qkvT2", qkvT[2]), ("d_kdup", kdup),
                            ("d_vS", vS), ("d_outT0", outT[0]),
                            ("d_outT1", outT[1])]:
                nc.sync.dma_start(out=dbg[nm], in_=src)

    nc.compile()
    return nc


def _get_nc():
    if "nc" not in _cache:
        _cache["nc"] = _build()
    return _cache["nc"]


def _in_maps(x, wq, bq, wk, bk, wv, bv, wo):
    import ml_dtypes

    bf = ml_dtypes.bfloat16
    x = np.asarray(x, np.float32)
    # xt[ni*128+p, dt*512+s'] = x[ni*512+s', dt*128+p]  (shared by all cores)
    xT = np.ascontiguousarray(x.T).astype(bf)                    # [d, s]
    xt = np.ascontiguousarray(
        xT.reshape(NT, 128, NI, 512).transpose(2, 1, 0, 3)
    ).reshape(NI * 128, NT * 512)
    maps = []
    for h in range(N_CORES):
        qs = slice(h * QF, (h + 1) * QF)
        ks = slice(h * DK, (h + 1) * DK)
        wqkv = np.concatenate([wq[qs], wk[ks], wv[ks]], axis=0)  # [384, 2048]
        wqkvT = np.ascontiguousarray(wqkv.T.astype(np.float32)).astype(bf)
        wt = np.ascontiguousarray(
            wqkvT.reshape(NT, 128, NF).transpose(1, 0, 2)
        ).reshape(128, NT * NF)
        woT = np.ascontiguousarray(wo[:, qs].T.astype(np.float32)).astype(bf)
        wob = np.ascontiguousarray(
            woT.reshape(2, 128, D).transpose(1, 0, 2)
        ).reshape(128, 2 * D)
        bqkv = np.concatenate([bq[qs], bk[ks], bv[ks]], axis=0).astype(np.float32)
        maps.append({
            "xt": xt,
            "wt": wt,
            "wob": wob,
            "bq": np.ascontiguousarray(bqkv[:, None]),
        })
    return maps


def _run(inputs, trace=False, tmpdir=None):
    from concourse.bass_utils import run_bass_kernel_spmd

    nc = _get_nc()
    x = np.asarray(inputs["x"])[0]
    maps = _in_maps(
        x,
        np.asarray(inputs["wq"]), np.asarray(inputs["bq"]),
        np.asarray(inputs["wk"]), np.asarray(inputs["bk"]),
        np.asarray(inputs["wv"]), np.asarray(inputs["bv"]),
        np.asarray(inputs["wo"]),
    )
    res = run_bass_kernel_spmd(
        nc, maps, list(range(N_CORES)), trace=trace, tmpdir=tmpdir
    )
    y = np.zeros((S, D), dtype=np.float32)
    for i in range(N_CORES):
        y += np.asarray(res.results[i]["y"]).astype(np.float32)
    y += np.asarray(inputs["bo"])[None, :]
    return y[None], res


def kernel(**inputs):
    y, _ = _run(inputs, trace=False)
    return y.astype(np.float32)


# revision 16
# speedup vs baseline: 1.5935x; 1.0674x over previous
"""GQA kernel for Trainium2, 8-way tensor-parallel over kv heads.

Problem (hardcoded): B=1, S=2048, D_MODEL=2048, HQ=32, HKV=8, DK=64, causal.
Sharding: core h owns kv head h and its 4 query heads. Weights are sliced,
transposed and cast to bf16 on host; x is replicated (transposed bf16). Each
core computes a partial y (its heads projected through its slice of wo); host
sums the 8 partials and adds bo.

On-chip dataflow per core (all matmuls bf16 with fp32 PSUM accumulation):
  xT, wqkvT, woT arrive pre-transposed -> no PE transposes for x/weights
  qkvT[f, s] projection, evacuated via DVE with fused per-partition bias
  scores: heads processed in pairs (g_even at array rows 0-63, g_odd at rows
    64-127 via duplicated K) -> the two 64-contraction matmuls run
    CONCURRENTLY in separate PE row groups (tile_position auto-derived)
  expT = exp(0.125 * scoresT) on ACT, causal strips only; diagonal blocks
    masked by gpsimd affine_select on the leading 128 columns only
  AV: col-tiled pair -- head g_even -> PSUM partitions 0-63, g_odd -> 64-127
    of one bank (concurrent), V is 64-wide; denominators via ones-column
    matmuls into partitions 0 / 32 of a second bank (concurrent pair)
  normalize: reciprocal_approx_fast + tiny broadcast matmuls + one DVE mul
  y = outT.T @ woT per 128-row block, interleaved into attention slack

Emission is software-pipelined with a filler queue: F(ti-1) and QKV(ti+1)
matmul chunks are popped between attention strips so PE stays busy while ACT
computes exp (ACT paces the attention phase).
"""

from collections import deque

import numpy as np

S = 2048
D = 2048
HQ, HKV, GRP, DK = 32, 8, 4, 64
QF = GRP * DK            # 256 query features per core
NF = QF + 2 * DK         # 384 projected features per core
N_CORES = 8
NT = S // 128            # 16 d-tiles
NI = S // 512            # 4 i-tiles

_cache = {}


def _build(debug_dumps=False):
    import concourse.bass as bass
    import concourse.mybir as mybir
    from concourse import bacc, tile
    from concourse.masks import make_identity
    from contextlib import ExitStack

    f32 = mybir.dt.float32
    bf16 = mybir.dt.bfloat16
    Exp = mybir.ActivationFunctionType.Exp

    nc = bacc.Bacc(
        "TRN2",
        target_bir_lowering=False,
        debug=False,
        enable_asserts=False,
        num_devices=N_CORES,
    )
    # host layouts (see _in_maps): xt[ni*128+p, dt*512+s'] = x[ni*512+s', dt*128+p]
    xt_d = nc.dram_tensor("xt", [NI * 128, NT * 512], bf16, kind="ExternalInput").ap()
    wt_d = nc.dram_tensor("wt", [128, NT * NF], bf16, kind="ExternalInput").ap()
    wo_d = nc.dram_tensor("wob", [128, 2 * D], bf16, kind="ExternalInput").ap()
    bq_d = nc.dram_tensor("bq", [NF, 1], f32, kind="ExternalInput").ap()
    y_d = nc.dram_tensor("y", [S, D], bf16, kind="ExternalOutput").ap()
    dbg = {}
    if debug_dumps:
        for nm, shp in [("d_qkvT0", [128, S]), ("d_qkvT1", [128, S]),
                        ("d_qkvT2", [128, S]), ("d_kdup", [128, S]),
                        ("d_vS", [128, NT * DK]), ("d_outT0", [128, S]),
                        ("d_outT1", [128, S])]:
            dbg[nm] = nc.dram_tensor(nm, shp, bf16, kind="ExternalOutput").ap()
        for nm, shp in [("d_aug", [128, 512]), ("d_dn", [33, 512]),
                        ("d_bcS", [128, 512])]:
            dbg[nm] = nc.dram_tensor(nm, shp, f32, kind="ExternalOutput").ap()

    with tile.TileContext(nc) as tc, ExitStack() as ctx:
        const = ctx.enter_context(tc.tile_pool(name="const", bufs=1))
        expp = ctx.enter_context(tc.tile_pool(name="expp", bufs=4))
        ysbp = ctx.enter_context(tc.tile_pool(name="ysbp", bufs=4))
        rcpp = ctx.enter_context(tc.tile_pool(name="rcpp", bufs=2))
        # PSUM (8 banks): sc 2x[128,1024]=4, av 1, dn/bc 1, qkv 1, f 1
        ps_sc = ctx.enter_context(tc.tile_pool(name="ps_sc", bufs=2, space="PSUM"))
        ps_av = ctx.enter_context(tc.tile_pool(name="ps_av", bufs=1, space="PSUM"))
        ps_dn = ctx.enter_context(tc.tile_pool(name="ps_dn", bufs=1, space="PSUM"))
        ps_qkv = ctx.enter_context(tc.tile_pool(name="ps_qkv", bufs=1, space="PSUM"))
        ps_f = ctx.enter_context(tc.tile_pool(name="ps_f", bufs=1, space="PSUM"))

        # ---- constants ----
        ident_bf = const.tile([128, 128], bf16)
        make_identity(nc, ident_bf)
        onescol = const.tile([128, 1], bf16)
        nc.gpsimd.memset(onescol, 1.0)
        onesB = const.tile([33, 64], bf16)
        nc.gpsimd.memset(onesB, 1.0)

        # ---- persistent SBUF ----
        XT = const.tile([128, NI * NT * 512], bf16)
        XTv = XT.rearrange("p (n t s) -> p n t s", n=NI, t=NT)
        wT = const.tile([128, NT * NF], bf16)
        wTv = wT.rearrange("p (t f) -> p t f", t=NT)
        woT = const.tile([128, 2 * D], bf16)
        woTv = woT.rearrange("p (t m) -> p t m", t=2)
        # qkvT[0] = Q heads g0|g1 (rows 0-63|64-127), [1] = g2|g3, [2] = K|V
        qkvT = [const.tile([128, S], bf16, name=f"qkvT{i}") for i in range(3)]
        kdup = const.tile([128, S], bf16)           # K duplicated at rows 64-127
        vS = const.tile([128, NT * DK], bf16)       # V as [s%128, strip, f]
        vSv = vS.rearrange("p (j f) -> p j f", j=NT)
        outT = [const.tile([128, S], bf16, name=f"outT{i}") for i in range(2)]
        btl = [const.tile([128, 1], f32, name=f"btl{i}") for i in range(3)]

        # ---- upfront DMAs (x slabs ordered by first use) ----
        nc.sync.dma_start(out=XTv[:, 0], in_=xt_d[0:128, :])
        nc.sync.dma_start(out=wT, in_=wt_d)
        for mi in range(3):
            nc.sync.dma_start(out=btl[mi], in_=bq_d[mi * 128:(mi + 1) * 128, :])
        nc.sync.dma_start(out=XTv[:, 1], in_=xt_d[128:256, :])
        nc.sync.dma_start(out=woT, in_=wo_d)
        nc.sync.dma_start(out=XTv[:, 2], in_=xt_d[256:384, :])
        nc.sync.dma_start(out=XTv[:, 3], in_=xt_d[384:512, :])

        # ---- stage emitters ----
        psq_live = {}

        def emit_qkv_chunk(nis, mi, c):
            # 4 of the 16 contraction tiles of the [128f, 512s] projection;
            # paired nis share the stationary weights (walrus dedupes the
            # LDWEIGHTS of consecutive same-lhsT matmuls)
            if c == 0:
                for ni in nis:
                    psq_live[(ni, mi)] = ps_qkv.tile(
                        [128, 512], f32, tag="ps_qkv", name="psq")
            for dt in range(4 * c, 4 * c + 4):
                for ni in nis:
                    nc.tensor.matmul(
                        psq_live[(ni, mi)],
                        lhsT=wTv[:, dt, mi * 128:(mi + 1) * 128],
                        rhs=XTv[:, ni, dt, :],
                        start=(dt == 0),
                        stop=(dt == NT - 1),
                    )
            if c == 3:
                for ni in nis:
                    psq = psq_live.pop((ni, mi))
                    nc.vector.tensor_scalar_add(
                        qkvT[mi][:, ni * 512:(ni + 1) * 512], psq, btl[mi])
                    if mi == 2:
                        nc.sync.dma_start(
                            out=kdup[64:128, ni * 512:(ni + 1) * 512],
                            in_=qkvT[2][0:DK, ni * 512:(ni + 1) * 512],
                        )

        def emit_vt(ni):
            # V strips of this i-tile transposed into vS via PE
            psv = ps_f.tile([128, 4 * DK], bf16, tag="ps_f", name="psv")
            for j in range(4):
                nc.tensor.transpose(
                    psv[:, j * DK:(j + 1) * DK],
                    qkvT[2][64:128, (4 * ni + j) * 128:(4 * ni + j + 1) * 128],
                    ident_bf[64:128, 64:128],
                )
            nc.vector.tensor_copy(
                vSv[:, 4 * ni:4 * ni + 4, :],
                psv.rearrange("p (a b) -> p a b", a=4),
            )

        def emit_f(sb, mi2):
            psy = ps_f.tile([128, 512], f32, tag="ps_f", name="psy")
            for ft in range(2):
                nc.tensor.matmul(
                    psy,
                    lhsT=outT[ft][:, sb * 128:(sb + 1) * 128],
                    rhs=woTv[:, ft, mi2 * 512:(mi2 + 1) * 512],
                    start=(ft == 0),
                    stop=(ft == 1),
                )
            ysb = ysbp.tile([128, 512], bf16, tag="ysbp", name="ysb")
            nc.vector.tensor_copy(ysb, psy)
            nc.sync.dma_start(
                out=y_d[sb * 128:(sb + 1) * 128, mi2 * 512:(mi2 + 1) * 512],
                in_=ysb,
            )

        def emit_scores(ti, p, bj):
            # head pair p: g_even at rows 0-63 (K source qkvT[2]), g_odd at
            # rows 64-127 (kdup). The two matmuls auto-derive tile_position
            # (0,0)/(64,0) -> they run concurrently in separate PE row groups.
            # Strips are computed full-width; causally dead columns are zeroed
            # after exp (keeps every PSUM byte initialized for the single exp).
            k = bj - 4 * ti
            off = 128 * k if k > 0 else 0
            W = 512 - off
            sc = ps_sc.tile([128, 1024], f32, tag="ps_sc", name="sc")
            # E half stored i-aligned at [off:512], O half packed at [512:512+W]
            # so the exp range [off:512+W] is contiguous and fully written
            nc.tensor.matmul(
                sc[:, off:512],
                lhsT=qkvT[2][0:DK, bj * 128:(bj + 1) * 128],
                rhs=qkvT[p][0:DK, ti * 512 + off:(ti + 1) * 512],
                start=True, stop=True,
            )
            nc.tensor.matmul(
                sc[:, 512:512 + W],
                lhsT=kdup[64:128, bj * 128:(bj + 1) * 128],
                rhs=qkvT[p][64:128, ti * 512 + off:(ti + 1) * 512],
                start=True, stop=True,
            )
            expT = expp.tile([128, 1024], bf16, tag="expp", name="expT")
            nc.scalar.activation(expT[:, off:512 + W], sc[:, off:512 + W],
                                 Exp, scale=0.125)
            if k >= 0:
                # zero j > i in the leading 128-col diagonal block of each half
                for lo in (off, 512):
                    nc.gpsimd.affine_select(
                        out=expT[:, lo:lo + 128],
                        in_=expT[:, lo:lo + 128],
                        compare_op=mybir.AluOpType.is_ge,
                        fill=0.0, base=0,
                        pattern=[[1, 128]], channel_multiplier=-1,
                    )
            return expT, off, W

        def emit_avdn(bj, nstr, expT, off, W, aug, dn):
            first, last = bj == 0, bj == nstr - 1
            # col-tiled AV pair: g_even -> psum partitions 0-63, g_odd -> 64-127
            nc.tensor.matmul(
                aug[0:DK, off:512], lhsT=vSv[:, bj, :], rhs=expT[:, off:512],
                start=first, stop=last, skip_group_check=True,
            )
            nc.tensor.matmul(
                aug[DK:128, off:512], lhsT=vSv[:, bj, :],
                rhs=expT[:, 512:512 + W],
                start=first, stop=last, skip_group_check=True,
            )
            # denominators: ones-column matmuls -> partitions 0 / 32 (concurrent)
            nc.tensor.matmul(
                dn[0:1, off:512], lhsT=onescol, rhs=expT[:, off:512],
                start=first, stop=last, skip_group_check=True,
            )
            nc.tensor.matmul(
                dn[32:33, off:512], lhsT=onescol, rhs=expT[:, 512:512 + W],
                start=first, stop=last, skip_group_check=True,
            )

        def emit_norm(ti, p, aug, dn):
            if debug_dumps and ti == 3 and p == 1:
                daug = const.tile([128, 512], f32, name="daug")
                nc.vector.tensor_copy(daug, aug)
                nc.sync.dma_start(out=dbg["d_aug"], in_=daug)
                ddn = const.tile([33, 512], f32, name="ddn")
                nc.vector.tensor_copy(ddn[0:1, :], dn[0:1, :])
                nc.vector.tensor_copy(ddn[32:33, :], dn[32:33, :])
                nc.sync.dma_start(out=dbg["d_dn"], in_=ddn)
            rcp = rcpp.tile([33, 512], f32, tag="rcp", name="rcp")
            rcpB = rcpp.tile([33, 512], bf16, tag="rcpB", name="rcpB")
            # full-tile op: reciprocal_approx_fast mis-executes on HW for APs
            # with base partition != 0 (probed), so cover rows 0..32 in one op
            # (rows 1-31 are junk-in/junk-out, initialized once below)
            nc.vector.reciprocal_approx_fast(rcp, dn[0:33, :])
            nc.vector.tensor_copy(rcpB, rcp)
            # broadcast recips over the pair's rows, reusing dn's bank
            nc.tensor.matmul(dn[0:DK, :], lhsT=onesB[0:1, :], rhs=rcpB[0:1, :],
                             start=True, stop=True, skip_group_check=True)
            nc.tensor.matmul(dn[DK:128, :], lhsT=onesB[32:33, :],
                             rhs=rcpB[32:33, :], start=True, stop=True,
                             skip_group_check=True)
            # DVE may read only one PSUM operand: stage bc in SBUF via ACT
            bcS = rcpp.tile([128, 512], bf16, tag="bcS", name="bcS")
            nc.scalar.copy(bcS, dn)
            if debug_dumps and ti == 3 and p == 1:
                dbcS = const.tile([128, 512], f32, name="dbcS")
                nc.vector.tensor_copy(dbcS, bcS)
                nc.sync.dma_start(out=dbg["d_bcS"], in_=dbcS)
            nc.vector.tensor_mul(outT[p][:, ti * 512:(ti + 1) * 512], aug, bcS)

        # ---- pipelined schedule ----
        fill = deque()

        def pump(n):
            for _ in range(n):
                if not fill:
                    return
                fill.popleft()()

        for mi in range(3):
            for c in range(4):
                emit_qkv_chunk((0,), mi, c)
        emit_vt(0)

        for ti in range(NI):
            if ti < NI - 1:
                for mi in range(3):
                    for c in range(4):
                        fill.append(lambda ni=ti + 1, mi=mi, c=c:
                                    emit_qkv_chunk((ni,), mi, c))
                fill.append(lambda ni=ti + 1: emit_vt(ni))
            if ti >= 1:
                for sb in range(4 * (ti - 1), 4 * ti):
                    for mi2 in range(4):
                        fill.append(lambda sb=sb, mi2=mi2: emit_f(sb, mi2))
            nstr = 4 * ti + 4
            for p in range(2):
                aug = ps_av.tile([128, 512], f32, tag="ps_av", name="aug")
                dn = ps_dn.tile([128, 512], f32, tag="ps_dn", name="dn")
                # keep rows 1-31 finite/nonzero and owned by this tile for the
                # full-tile reciprocal (row 0 is overwritten by the start=True
                # denominator matmul)
                nc.vector.memset(dn[0:32, :], 1.0)
                prev = None
                for bj in range(nstr):
                    cur = emit_scores(ti, p, bj)
                    pump(1)
                    if prev is not None:
                        emit_avdn(bj - 1, nstr, *prev, aug, dn)
                    prev = cur
                pump(1)
                emit_avdn(nstr - 1, nstr, *prev, aug, dn)
                emit_norm(ti, p, aug, dn)
            pump(len(fill))
        # tail: attention banks are idle now -- rotate the final F through the
        # freed ps_sc slots with alternating evac engines so the PE stays
        # dense (and HAM stays warm) to the end
        for i, (sb, mi2) in enumerate(
                (sb, mi2) for sb in range(4 * (NI - 1), 4 * NI)
                for mi2 in range(4)):
            psy = ps_sc.tile([128, 512], f32, tag="ps_sc", name="psyt")
            for ft in range(2):
                nc.tensor.matmul(
                    psy,
                    lhsT=outT[ft][:, sb * 128:(sb + 1) * 128],
                    rhs=woTv[:, ft, mi2 * 512:(mi2 + 1) * 512],
                    start=(ft == 0),
                    stop=(ft == 1),
                )
            ysb = ysbp.tile([128, 512], bf16, tag="ysbp", name="ysb")
            if i % 2 == 0:
                nc.vector.tensor_copy(ysb, psy)
            else:
                nc.scalar.copy(ysb, psy)
            nc.sync.dma_start(
                out=y_d[sb * 128:(sb + 1) * 128, mi2 * 512:(mi2 + 1) * 512],
                in_=ysb,
            )

        if debug_dumps:
            for nm, src in [("d_qkvT0", qkvT[0]), ("d_qkvT1", qkvT[1]),
                            ("d_qkvT2", qkvT[2]), ("d_kdup", kdup),
                            ("d_vS", vS), ("d_outT0", outT[0]),
                            ("d_outT1", outT[1])]:
                nc.sync.dma_start(out=dbg[nm], in_=src)

    nc.compile()
    return nc


def _get_nc():
    if "nc" not in _cache:
        _cache["nc"] = _build()
    return _cache["nc"]


def _in_maps(x, wq, bq, wk, bk, wv, bv, wo):
    import ml_dtypes

    bf = ml_dtypes.bfloat16
    x = np.asarray(x, np.float32)
    # xt[ni*128+p, dt*512+s'] = x[ni*512+s', dt*128+p]  (shared by all cores)
    xT = np.ascontiguousarray(x.T).astype(bf)                    # [d, s]
    xt = np.ascontiguousarray(
        xT.reshape(NT, 128, NI, 512).transpose(2, 1, 0, 3)
    ).reshape(NI * 128, NT * 512)
    maps = []
    for h in range(N_CORES):
        qs = slice(h * QF, (h + 1) * QF)
        ks = slice(h * DK, (h + 1) * DK)
        wqkv = np.concatenate([wq[qs], wk[ks], wv[ks]], axis=0)  # [384, 2048]
        wqkvT = np.ascontiguousarray(wqkv.T.astype(np.float32)).astype(bf)
        wt = np.ascontiguousarray(
            wqkvT.reshape(NT, 128, NF).transpose(1, 0, 2)
        ).reshape(128, NT * NF)
        woT = np.ascontiguousarray(wo[:, qs].T.astype(np.float32)).astype(bf)
        wob = np.ascontiguousarray(
            woT.reshape(2, 128, D).transpose(1, 0, 2)
        ).reshape(128, 2 * D)
        bqkv = np.concatenate([bq[qs], bk[ks], bv[ks]], axis=0).astype(np.float32)
        maps.append({
            "xt": xt,
            "wt": wt,
            "wob": wob,
            "bq": np.ascontiguousarray(bqkv[:, None]),
        })
    return maps


def _run(inputs, trace=False, tmpdir=None):
    from concourse.bass_utils import run_bass_kernel_spmd

    nc = _get_nc()
    x = np.asarray(inputs["x"])[0]
    maps = _in_maps(
        x,
        np.asarray(inputs["wq"]), np.asarray(inputs["bq"]),
        np.asarray(inputs["wk"]), np.asarray(inputs["bk"]),
        np.asarray(inputs["wv"]), np.asarray(inputs["bv"]),
        np.asarray(inputs["wo"]),
    )
    res = run_bass_kernel_spmd(
        nc, maps, list(range(N_CORES)), trace=trace, tmpdir=tmpdir
    )
    y = np.zeros((S, D), dtype=np.float32)
    for i in range(N_CORES):
        y += np.asarray(res.results[i]["y"]).astype(np.float32)
    y += np.asarray(inputs["bo"])[None, :]
    return y[None], res


def kernel(**inputs):
    y, _ = _run(inputs, trace=False)
    return y.astype(np.float32)


# revision 23
# speedup vs baseline: 1.6284x; 1.0219x over previous
"""GQA kernel for Trainium2, 8-way tensor-parallel over kv heads.

Problem (hardcoded): B=1, S=2048, D_MODEL=2048, HQ=32, HKV=8, DK=64, causal.
Sharding: core h owns kv head h and its 4 query heads. Weights are sliced,
transposed and cast to bf16 on host; x is replicated (transposed bf16). Each
core computes a partial y (its heads projected through its slice of wo); host
sums the 8 partials and adds bo.

On-chip dataflow per core (all matmuls bf16 with fp32 PSUM accumulation):
  xT, wqkvT, woT arrive pre-transposed -> no PE transposes for x/weights
  qkvT[f, s] projection, evacuated via DVE with fused per-partition bias
  scores: heads processed in pairs (g_even at array rows 0-63, g_odd at rows
    64-127 via duplicated K) -> the two 64-contraction matmuls run
    CONCURRENTLY in separate PE row groups (tile_position auto-derived)
  expT = exp(0.125 * scoresT) on ACT, causal strips only; diagonal blocks
    masked by gpsimd affine_select on the leading 128 columns only
  AV: col-tiled pair -- head g_even -> PSUM partitions 0-63, g_odd -> 64-127
    of one bank (concurrent), V is 64-wide; denominators via ones-column
    matmuls into partitions 0 / 32 of a second bank (concurrent pair)
  normalize: reciprocal_approx_fast + tiny broadcast matmuls + one DVE mul
  y = outT.T @ woT per 128-row block, interleaved into attention slack

Emission is software-pipelined with a filler queue: F(ti-1) and QKV(ti+1)
matmul chunks are popped between attention strips so PE stays busy while ACT
computes exp (ACT paces the attention phase).
"""

from collections import deque

import numpy as np

S = 2048
D = 2048
HQ, HKV, GRP, DK = 32, 8, 4, 64
QF = GRP * DK            # 256 query features per core
NF = QF + 2 * DK         # 384 projected features per core
N_CORES = 8
NT = S // 128            # 16 d-tiles
NI = S // 512            # 4 i-tiles

_cache = {}


def _build(debug_dumps=False):
    import concourse.bass as bass
    import concourse.mybir as mybir
    from concourse import bacc, tile
    from concourse.masks import make_identity
    from contextlib import ExitStack

    f32 = mybir.dt.float32
    bf16 = mybir.dt.bfloat16
    Exp = mybir.ActivationFunctionType.Exp

    nc = bacc.Bacc(
        "TRN2",
        target_bir_lowering=False,
        debug=False,
        enable_asserts=False,
        num_devices=N_CORES,
    )
    # host layouts (see _in_maps): xt[ni*128+p, dt*512+s'] = x[ni*512+s', dt*128+p]
    xt_d = nc.dram_tensor("xt", [NI * 128, NT * 512], bf16, kind="ExternalInput").ap()
    wt_d = nc.dram_tensor("wt", [128, NT * NF], bf16, kind="ExternalInput").ap()
    wo_d = nc.dram_tensor("wob", [128, 2 * D], bf16, kind="ExternalInput").ap()
    bq_d = nc.dram_tensor("bq", [NF, 1], f32, kind="ExternalInput").ap()
    y_d = nc.dram_tensor("y", [S, D], bf16, kind="ExternalOutput").ap()
    dbg = {}
    if debug_dumps:
        for nm, shp in [("d_qkvT0", [128, S]), ("d_qkvT1", [128, S]),
                        ("d_qkvT2", [128, S]), ("d_kdup", [128, S]),
                        ("d_vS", [128, NT * DK]), ("d_outT0", [128, S]),
                        ("d_outT1", [128, S])]:
            dbg[nm] = nc.dram_tensor(nm, shp, bf16, kind="ExternalOutput").ap()
        for nm, shp in [("d_aug", [128, 512]), ("d_dn", [33, 512]),
                        ("d_bcS", [128, 512])]:
            dbg[nm] = nc.dram_tensor(nm, shp, f32, kind="ExternalOutput").ap()

    with tile.TileContext(nc) as tc, ExitStack() as ctx:
        const = ctx.enter_context(tc.tile_pool(name="const", bufs=1))
        expp = ctx.enter_context(tc.tile_pool(name="expp", bufs=4))
        ysbp = ctx.enter_context(tc.tile_pool(name="ysbp", bufs=4))
        rcpp = ctx.enter_context(tc.tile_pool(name="rcpp", bufs=2))
        # PSUM (8 banks): sc 2x[128,1024]=4, av 1, dn/bc 1, qkv 1, f 1
        ps_sc = ctx.enter_context(tc.tile_pool(name="ps_sc", bufs=2, space="PSUM"))
        ps_av = ctx.enter_context(tc.tile_pool(name="ps_av", bufs=1, space="PSUM"))
        ps_dn = ctx.enter_context(tc.tile_pool(name="ps_dn", bufs=1, space="PSUM"))
        ps_qkv = ctx.enter_context(tc.tile_pool(name="ps_qkv", bufs=1, space="PSUM"))
        ps_f = ctx.enter_context(tc.tile_pool(name="ps_f", bufs=1, space="PSUM"))

        # ---- constants ----
        ident_bf = const.tile([128, 128], bf16)
        make_identity(nc, ident_bf)
        onescol = const.tile([128, 1], bf16)
        nc.gpsimd.memset(onescol, 1.0)
        onesB = const.tile([33, 64], bf16)
        nc.gpsimd.memset(onesB, 1.0)

        # ---- persistent SBUF ----
        XT = const.tile([128, NI * NT * 512], bf16)
        XTv = XT.rearrange("p (n t s) -> p n t s", n=NI, t=NT)
        wT = const.tile([128, NT * NF], bf16)
        wTv = wT.rearrange("p (t f) -> p t f", t=NT)
        woT = const.tile([128, 2 * D], bf16)
        woTv = woT.rearrange("p (t m) -> p t m", t=2)
        # qkvT[0] = Q heads g0|g1 (rows 0-63|64-127), [1] = g2|g3, [2] = K|V
        qkvT = [const.tile([128, S], bf16, name=f"qkvT{i}") for i in range(3)]
        kdup = const.tile([128, S], bf16)           # K duplicated at rows 64-127
        vS = const.tile([128, NT * DK], bf16)       # V as [s%128, strip, f]
        vSv = vS.rearrange("p (j f) -> p j f", j=NT)
        outT = [const.tile([128, S], bf16, name=f"outT{i}") for i in range(2)]
        btl = [const.tile([128, 1], f32, name=f"btl{i}") for i in range(3)]

        # ---- upfront DMAs (x slabs ordered by first use; the ni=0 slab and
        # wT arrive in dt-quad chunks so QKV(0)'s first matmuls start early) ----
        wTq = wT.rearrange("p (q r) -> p q r", q=4)
        for q in range(4):
            nc.sync.dma_start(out=XTv[:, 0, 4 * q:4 * q + 4, :],
                              in_=xt_d[0:128, q * 2048:(q + 1) * 2048])
            nc.sync.dma_start(out=wTq[:, q],
                              in_=wt_d[:, q * 4 * NF:(q + 1) * 4 * NF])
        for mi in range(3):
            nc.sync.dma_start(out=btl[mi], in_=bq_d[mi * 128:(mi + 1) * 128, :])
        nc.sync.dma_start(out=XTv[:, 1], in_=xt_d[128:256, :])
        nc.sync.dma_start(out=woT, in_=wo_d)
        nc.sync.dma_start(out=XTv[:, 2], in_=xt_d[256:384, :])
        nc.sync.dma_start(out=XTv[:, 3], in_=xt_d[384:512, :])

        # ---- stage emitters ----
        psq_live = {}

        def emit_qkv_chunk(nis, mi, c, pool=None, tg="ps_qkv"):
            # 4 of the 16 contraction tiles of the [128f, 512s] projection;
            # paired nis share the stationary weights (walrus dedupes the
            # LDWEIGHTS of consecutive same-lhsT matmuls)
            if c == 0:
                for ni in nis:
                    psq_live[(ni, mi)] = (pool or ps_qkv).tile(
                        [128, 512], f32, tag=tg, name="psq")
            for dt in range(4 * c, 4 * c + 4):
                for ni in nis:
                    nc.tensor.matmul(
                        psq_live[(ni, mi)],
                        lhsT=wTv[:, dt, mi * 128:(mi + 1) * 128],
                        rhs=XTv[:, ni, dt, :],
                        start=(dt == 0),
                        stop=(dt == NT - 1),
                    )
            if c == 3:
                for ni in nis:
                    psq = psq_live.pop((ni, mi))
                    nc.vector.tensor_scalar_add(
                        qkvT[mi][:, ni * 512:(ni + 1) * 512], psq, btl[mi])
                    if mi == 2:
                        nc.sync.dma_start(
                            out=kdup[64:128, ni * 512:(ni + 1) * 512],
                            in_=qkvT[2][0:DK, ni * 512:(ni + 1) * 512],
                        )

        def emit_vt(ni):
            # V strips of this i-tile transposed into vS via PE
            psv = ps_f.tile([128, 4 * DK], bf16, tag="ps_f", name="psv")
            for j in range(4):
                nc.tensor.transpose(
                    psv[:, j * DK:(j + 1) * DK],
                    qkvT[2][64:128, (4 * ni + j) * 128:(4 * ni + j + 1) * 128],
                    ident_bf[64:128, 64:128],
                )
            nc.vector.tensor_copy(
                vSv[:, 4 * ni:4 * ni + 4, :],
                psv.rearrange("p (a b) -> p a b", a=4),
            )

        def emit_f(sb, mi2):
            psy = ps_f.tile([128, 512], f32, tag="ps_f", name="psy")
            for ft in range(2):
                nc.tensor.matmul(
                    psy,
                    lhsT=outT[ft][:, sb * 128:(sb + 1) * 128],
                    rhs=woTv[:, ft, mi2 * 512:(mi2 + 1) * 512],
                    start=(ft == 0),
                    stop=(ft == 1),
                )
            ysb = ysbp.tile([128, 512], bf16, tag="ysbp", name="ysb")
            nc.vector.tensor_copy(ysb, psy)
            nc.sync.dma_start(
                out=y_d[sb * 128:(sb + 1) * 128, mi2 * 512:(mi2 + 1) * 512],
                in_=ysb,
            )

        def emit_scores(ti, p, bj):
            # head pair p: g_even at rows 0-63 (K source qkvT[2]), g_odd at
            # rows 64-127 (kdup). The two matmuls auto-derive tile_position
            # (0,0)/(64,0) -> they run concurrently in separate PE row groups.
            # Strips are computed full-width; causally dead columns are zeroed
            # after exp (keeps every PSUM byte initialized for the single exp).
            k = bj - 4 * ti
            off = 128 * k if k > 0 else 0
            W = 512 - off
            sc = ps_sc.tile([128, 1024], f32, tag="ps_sc", name="sc")
            # E half stored i-aligned at [off:512], O half packed at [512:512+W]
            # so the exp range [off:512+W] is contiguous and fully written
            nc.tensor.matmul(
                sc[:, off:512],
                lhsT=qkvT[2][0:DK, bj * 128:(bj + 1) * 128],
                rhs=qkvT[p][0:DK, ti * 512 + off:(ti + 1) * 512],
                start=True, stop=True,
            )
            nc.tensor.matmul(
                sc[:, 512:512 + W],
                lhsT=kdup[64:128, bj * 128:(bj + 1) * 128],
                rhs=qkvT[p][64:128, ti * 512 + off:(ti + 1) * 512],
                start=True, stop=True,
            )
            expT = expp.tile([128, 1024], bf16, tag="expp", name="expT")
            nc.scalar.activation(expT[:, off:512 + W], sc[:, off:512 + W],
                                 Exp, scale=0.125)
            if k >= 0:
                # zero j > i in the leading 128-col diagonal block of each half
                for lo in (off, 512):
                    nc.gpsimd.affine_select(
                        out=expT[:, lo:lo + 128],
                        in_=expT[:, lo:lo + 128],
                        compare_op=mybir.AluOpType.is_ge,
                        fill=0.0, base=0,
                        pattern=[[1, 128]], channel_multiplier=-1,
                    )
            return expT, off, W

        def emit_avdn(bj, nstr, expT, off, W, aug, dn):
            first, last = bj == 0, bj == nstr - 1
            # col-tiled AV pair: g_even -> psum partitions 0-63, g_odd -> 64-127
            nc.tensor.matmul(
                aug[0:DK, off:512], lhsT=vSv[:, bj, :], rhs=expT[:, off:512],
                start=first, stop=last, skip_group_check=True,
            )
            nc.tensor.matmul(
                aug[DK:128, off:512], lhsT=vSv[:, bj, :],
                rhs=expT[:, 512:512 + W],
                start=first, stop=last, skip_group_check=True,
            )
            # denominators: ones-column matmuls -> partitions 0 / 32 (concurrent)
            nc.tensor.matmul(
                dn[0:1, off:512], lhsT=onescol, rhs=expT[:, off:512],
                start=first, stop=last, skip_group_check=True,
            )
            nc.tensor.matmul(
                dn[32:33, off:512], lhsT=onescol, rhs=expT[:, 512:512 + W],
                start=first, stop=last, skip_group_check=True,
            )

        def emit_norm(ti, p, aug, dn):
            if debug_dumps and ti == 3 and p == 1:
                daug = const.tile([128, 512], f32, name="daug")
                nc.vector.tensor_copy(daug, aug)
                nc.sync.dma_start(out=dbg["d_aug"], in_=daug)
                ddn = const.tile([33, 512], f32, name="ddn")
                nc.vector.tensor_copy(ddn[0:1, :], dn[0:1, :])
                nc.vector.tensor_copy(ddn[32:33, :], dn[32:33, :])
                nc.sync.dma_start(out=dbg["d_dn"], in_=ddn)
            rcp = rcpp.tile([33, 512], f32, tag="rcp", name="rcp")
            rcpB = rcpp.tile([33, 512], bf16, tag="rcpB", name="rcpB")
            # full-tile op: reciprocal_approx_fast mis-executes on HW for APs
            # with base partition != 0 (probed), so cover rows 0..32 in one op
            # (rows 1-31 are junk-in/junk-out, initialized once below)
            nc.vector.reciprocal_approx_fast(rcp, dn[0:33, :])
            nc.vector.tensor_copy(rcpB, rcp)
            # broadcast recips over the pair's rows, reusing dn's bank
            nc.tensor.matmul(dn[0:DK, :], lhsT=onesB[0:1, :], rhs=rcpB[0:1, :],
                             start=True, stop=True, skip_group_check=True)
            nc.tensor.matmul(dn[DK:128, :], lhsT=onesB[32:33, :],
                             rhs=rcpB[32:33, :], start=True, stop=True,
                             skip_group_check=True)
            # DVE may read only one PSUM operand: stage bc in SBUF via ACT
            bcS = rcpp.tile([128, 512], bf16, tag="bcS", name="bcS")
            nc.scalar.copy(bcS, dn)
            if debug_dumps and ti == 3 and p == 1:
                dbcS = const.tile([128, 512], f32, name="dbcS")
                nc.vector.tensor_copy(dbcS, bcS)
                nc.sync.dma_start(out=dbg["d_bcS"], in_=dbcS)
            nc.vector.tensor_mul(outT[p][:, ti * 512:(ti + 1) * 512], aug, bcS)

        # ---- pipelined schedule ----
        fill = deque()

        def pump(n):
            for _ in range(n):
                if not fill:
                    return
                fill.popleft()()

        # startup QKV(0) rotates through the (still idle) score banks so the
        # three mi-groups never stall on a single bank's evacuation
        for mi in range(3):
            for c in range(4):
                emit_qkv_chunk((0,), mi, c, pool=ps_sc, tg="ps_sc")
        emit_vt(0)

        for ti in range(NI):
            if ti < NI - 1:
                for mi in range(3):
                    for c in range(4):
                        fill.append(lambda ni=ti + 1, mi=mi, c=c:
                                    emit_qkv_chunk((ni,), mi, c))
                fill.append(lambda ni=ti + 1: emit_vt(ni))
            if ti >= 1:
                for sb in range(4 * (ti - 1), 4 * ti):
                    for mi2 in range(4):
                        fill.append(lambda sb=sb, mi2=mi2: emit_f(sb, mi2))
            nstr = 4 * ti + 4
            for p in range(2):
                aug = ps_av.tile([128, 512], f32, tag="ps_av", name="aug")
                dn = ps_dn.tile([128, 512], f32, tag="ps_dn", name="dn")
                # keep rows 1-31 finite/nonzero and owned by this tile for the
                # full-tile reciprocal (row 0 is overwritten by the start=True
                # denominator matmul)
                nc.vector.memset(dn[0:32, :], 1.0)
                prev = None
                for bj in range(nstr):
                    cur = emit_scores(ti, p, bj)
                    pump(1)
                    if prev is not None:
                        emit_avdn(bj - 1, nstr, *prev, aug, dn)
                    prev = cur
                pump(1)
                emit_avdn(nstr - 1, nstr, *prev, aug, dn)
                emit_norm(ti, p, aug, dn)
            pump(len(fill))
        # tail: attention banks are idle now -- rotate the final F through the
        # freed ps_sc/av/dn banks with alternating evac engines so the PE
        # stays dense (and HAM stays warm) to the end
        tail_pools = [(ps_sc, "ps_sc"), (ps_av, "ps_av"),
                      (ps_sc, "ps_sc"), (ps_dn, "ps_dn")]
        for i, (sb, mi2) in enumerate(
                (sb, mi2) for sb in range(4 * (NI - 1), 4 * NI)
                for mi2 in range(4)):
            pool, tg = tail_pools[i % 4]
            psy = pool.tile([128, 512], f32, tag=tg, name="psyt")
            for ft in range(2):
                nc.tensor.matmul(
                    psy,
                    lhsT=outT[ft][:, sb * 128:(sb + 1) * 128],
                    rhs=woTv[:, ft, mi2 * 512:(mi2 + 1) * 512],
                    start=(ft == 0),
                    stop=(ft == 1),
                )
            ysb = ysbp.tile([128, 512], bf16, tag="ysbp", name="ysb")
            if i % 2 == 0:
                nc.vector.tensor_copy(ysb, psy)
            else:
                nc.scalar.copy(ysb, psy)
            nc.sync.dma_start(
                out=y_d[sb * 128:(sb + 1) * 128, mi2 * 512:(mi2 + 1) * 512],
                in_=ysb,
            )

        if debug_dumps:
            for nm, src in [("d_qkvT0", qkvT[0]), ("d_qkvT1", qkvT[1]),
                            ("d_qkvT2", qkvT[2]), ("d_kdup", kdup),
                            ("d_vS", vS), ("d_outT0", outT[0]),
                            ("d_outT1", outT[1])]:
                nc.sync.dma_start(out=dbg[nm], in_=src)

    nc.compile()
    return nc


def _get_nc():
    if "nc" not in _cache:
        _cache["nc"] = _build()
    return _cache["nc"]


def _in_maps(x, wq, bq, wk, bk, wv, bv, wo):
    import ml_dtypes

    bf = ml_dtypes.bfloat16
    x = np.asarray(x, np.float32)
    # xt[ni*128+p, dt*512+s'] = x[ni*512+s', dt*128+p]  (shared by all cores)
    xT = np.ascontiguousarray(x.T).astype(bf)                    # [d, s]
    xt = np.ascontiguousarray(
        xT.reshape(NT, 128, NI, 512).transpose(2, 1, 0, 3)
    ).reshape(NI * 128, NT * 512)
    maps = []
    for h in range(N_CORES):
        qs = slice(h * QF, (h + 1) * QF)
        ks = slice(h * DK, (h + 1) * DK)
        wqkv = np.concatenate([wq[qs], wk[ks], wv[ks]], axis=0)  # [384, 2048]
        wqkvT = np.ascontiguousarray(wqkv.T.astype(np.float32)).astype(bf)
        wt = np.ascontiguousarray(
            wqkvT.reshape(NT, 128, NF).transpose(1, 0, 2)
        ).reshape(128, NT * NF)
        woT = np.ascontiguousarray(wo[:, qs].T.astype(np.float32)).astype(bf)
        wob = np.ascontiguousarray(
            woT.reshape(2, 128, D).transpose(1, 0, 2)
        ).reshape(128, 2 * D)
        bqkv = np.concatenate([bq[qs], bk[ks], bv[ks]], axis=0).astype(np.float32)
        maps.append({
            "xt": xt,
            "wt": wt,
            "wob": wob,
            "bq": np.ascontiguousarray(bqkv[:, None]),
        })
    return maps


def _run(inputs, trace=False, tmpdir=None):
    from concourse.bass_utils import run_bass_kernel_spmd

    nc = _get_nc()
    x = np.asarray(inputs["x"])[0]
    maps = _in_maps(
        x,
        np.asarray(inputs["wq"]), np.asarray(inputs["bq"]),
        np.asarray(inputs["wk"]), np.asarray(inputs["bk"]),
        np.asarray(inputs["wv"]), np.asarray(inputs["bv"]),
        np.asarray(inputs["wo"]),
    )
    res = run_bass_kernel_spmd(
        nc, maps, list(range(N_CORES)), trace=trace, tmpdir=tmpdir
    )
    y = np.zeros((S, D), dtype=np.float32)
    for i in range(N_CORES):
        y += np.asarray(res.results[i]["y"]).astype(np.float32)
    y += np.asarray(inputs["bo"])[None, :]
    return y[None], res


def kernel(**inputs):
    y, _ = _run(inputs, trace=False)
    return y.astype(np.float32)
